# revision 23
# baseline (speedup 1.0000x reference)
"""Trainium2 Bass kernel for DiffKS (differentiable Karplus-Strong string).

Math (per sequence b, time n):
    g = 0.99*l_b[...,0]; p = l_b[...,1]
    b0 = g*(1-p); a1 = g*p
    f0c = f0 - a1/(b0+a1+1e-7)
    z = floor(f0c); zc = z-2; alpha = f0c - zc
    w_j = Lagrange weights (order 5), j=0..5
    block_j = b0*w_j + a1*w_{j-1}, j=0..6           (7 taps)
    taps live at k = c0+j, c0 = zc-1 = z-3 in [36, 96]
    y[n] = x[n] + sum_j block_j[n] * y[n-1-(c0[n]+j)]    (delays 37..103)

Key structure: minimum delay is 37 > 32, so 32-sample chunks are internally
parallel.  Chunk c is computed as accumulating PE matmuls against the previous
4 chunks' outputs, with per-chunk tap matrices built on-chip by a GPSIMD
local_scatter + DVE 32x32 block transpose.  B=16 is sharded 2 seqs/core.

Phase-1 optimized layout vs the original baseline:
  - both sequences share one interleaved ring tile ringI[128, 2*(NCH/4+1)]
    (col 2*nu+b) and one psum tile [128, 2] per chunk, so each chunk needs a
    single [32,2] DVE evac instead of two.
  - matmul pieces with contiguous rows are merged (avg 1.75 vs 2.25 per
    chunk per seq).
  - natural->S-plane transposes are done as 16 full 128x128 PE transposes
    plus 4-replication matmuls with shared stationary (Rep_rho), evacuated
    with strided copies split between DVE and ACT.
  - scatter index math is reduced (~42 ops/seq) and runs on GPSIMD,
    overlapped with the tap math / transposes, sliced so scatters start
    before all index math finishes.

Layouts (per core, seqs b=0,1; chunk T=32; NCH = N/32 chunks; NP = N/128):
  natural plane  nat[P, b*128+j]  = q[b, 128*P + j]          [NP, 256]
  S-plane        qS[32*rho+f, c]  = q[b, 32*c + f], c = 4P+rho (replicated
                 over rho for scatter source planes)          [128, NCH]
  ring           ringI[32*(c%4)+f, 2*(1+c//4)+b] = y[b, 32*c+f]
Tap matrix for chunk c (lhsT for the PE matmul): rows 32*fl + (31 - m)
address the ring window column; scatter writes single u16s of bf16 taps.

Phase-2 (this session): chain data in bf16 (taps + ring; psum accumulation
stays fp32) -> single-pass PE matmuls instead of fp32 LOW/HIGH, half the
scatter indices, 2x faster DVE transposes; chain evacs split DVE (seq 0) /
ACT (seq 1) so the two evacs run concurrently and ACT is off the DVE queue.
Verified offline: bf16 taps+ring gives ~2e-3 rel err (budget 2e-2).
"""

import numpy as np

import concourse.bass as bass
import concourse.mybir as mybir
import concourse.bacc as bacc
import concourse.tile as tile
from concourse import bass_utils

F32 = mybir.dt.float32
BF16 = mybir.dt.bfloat16
I32 = mybir.dt.int32
I16 = mybir.dt.int16
U16 = mybir.dt.uint16
AO = mybir.AluOpType
AF = mybir.ActivationFunctionType

B_FULL = 16
N_FULL = 16384
NCORES = 8
B_LOC = 2  # sequences per core
GS = 8     # chunks per scatter group

# matmul piece tables per t=c%4: (row_base, row_size, col_delta); ring column
# read is (c//4) + col_delta.  Contiguous same-col-delta rows are merged where
# tile_position allows (row base 0 for sizes > 64); the tile's row space is
# shared between col deltas, so pieces must never overlap rows.
# col base None = main region (32*s); "X" = extra region for t=1's c-1 piece
PIECES = {
    0: [(0, 128, 0, None)],
    1: [(0, 128, 0, None), (0, 32, 1, "X")],
    2: [(64, 64, 0, None), (0, 64, 1, None)],
    3: [(96, 32, 0, None), (0, 96, 1, None)],
}

# Lagrange denominators 1/d_j for order 5
INV_D = [-1.0 / 120, 1.0 / 24, -1.0 / 12, 1.0 / 12, -1.0 / 24, 1.0 / 120]


def build_kernel(tc, out_d, f0_d, x_d, lb_d, N):
    nc = tc.nc
    NP = N // 128          # natural-plane columns per seq
    NCH = N // 32          # chunks per seq
    NG = NCH // GS         # scatter groups
    assert NP * 128 == N and NP == 128 and NG * GS == NCH

    import contextlib
    ctx = contextlib.ExitStack()
    pp = ctx.enter_context(tc.tile_pool(name="persist", bufs=1))
    traw_pool = ctx.enter_context(tc.tile_pool(name="traw", bufs=4))
    t4_pool = ctx.enter_context(tc.tile_pool(name="t4", bufs=12))
    psum_t = ctx.enter_context(tc.tile_pool(name="psum_t", bufs=2, space="PSUM"))
    psum_r = psum_t
    psum_c = ctx.enter_context(tc.tile_pool(name="psum_c", bufs=3, space="PSUM"))

    V = nc.vector
    G = nc.gpsimd
    SC = nc.scalar

    with ctx:
        # ---------------- phase 0: load + elementwise tap math ----------------
        nat_f0 = pp.tile([NP, 256], F32)
        nat_x = pp.tile([NP, 256], F32)
        nat_lb = pp.tile([NP, 512], F32)
        for b in range(B_LOC):
            nc.sync.dma_start(
                out=nat_f0[:, b * 128:(b + 1) * 128],
                in_=f0_d[b].rearrange("(p j) -> p j", j=128),
            )
            nc.sync.dma_start(
                out=nat_x[:, b * 128:(b + 1) * 128],
                in_=x_d[b].rearrange("(p j) -> p j", j=128),
            )
            nc.sync.dma_start(
                out=nat_lb[:, b * 256:(b + 1) * 256],
                in_=lb_d[b].rearrange("(p j) s -> p (j s)", j=128),
            )
        lb_r = nat_lb[:].rearrange("p (j s) -> p j s", s=2)
        g_ap = lb_r[:, :, 0]
        p_ap = lb_r[:, :, 1]

        g99 = pp.tile([NP, 256], F32)
        t_gp = pp.tile([NP, 256], F32)   # a1 = 0.99*g*p
        b0t = pp.tile([NP, 256], F32)
        rec = pp.tile([NP, 256], F32)
        f0c = pp.tile([NP, 256], F32)
        zf = pp.tile([NP, 256], F32)
        tmp1 = pp.tile([NP, 256], F32)
        tmp2 = pp.tile([NP, 256], F32)
        itmp = pp.tile([NP, 256], I32)

        # zf chain first so its transposes (and gpsimd index math) start early
        V.tensor_scalar(out=g99[:], in0=g_ap, scalar1=0.99, scalar2=None, op0=AO.mult)
        V.tensor_tensor(out=t_gp[:], in0=g99[:], in1=p_ap, op=AO.mult)       # a1
        V.tensor_tensor(out=b0t[:], in0=g99[:], in1=t_gp[:], op=AO.subtract)  # b0
        V.tensor_scalar(out=tmp1[:], in0=g99[:], scalar1=1e-7, scalar2=None, op0=AO.add)
        V.reciprocal(out=rec[:], in_=tmp1[:])
        V.tensor_tensor(out=tmp2[:], in0=t_gp[:], in1=rec[:], op=AO.mult)
        V.tensor_tensor(out=f0c[:], in0=nat_f0[:], in1=tmp2[:], op=AO.subtract)
        # zf = floor(f0c), robust to cast rounding mode
        V.tensor_copy(out=itmp[:], in_=f0c[:])
        V.tensor_copy(out=zf[:], in_=itmp[:])
        V.tensor_tensor(out=tmp1[:], in0=zf[:], in1=f0c[:], op=AO.is_gt)
        V.tensor_tensor(out=zf[:], in0=zf[:], in1=tmp1[:], op=AO.subtract)

        # ---- transpose machinery (identity + replication stationaries) ----
        ident = pp.tile([128, 128], F32)
        G.memset(ident[:], 1.0)
        G.affine_select(out=ident[:], in_=ident[:], pattern=[[1, 128]],
                        compare_op=AO.is_equal, fill=0.0, base=0,
                        channel_multiplier=-1)
        ident4b = pp.tile([128, 32], BF16)
        G.memset(ident4b[:], 1.0)
        for g0 in range(4):
            G.affine_select(out=ident4b[32 * g0:32 * g0 + 32, :],
                            in_=ident4b[32 * g0:32 * g0 + 32, :],
                            pattern=[[1, 32]], compare_op=AO.is_equal,
                            fill=0.0, base=0, channel_multiplier=-1)
        # Rep[rho]: [128,128] with identity blocks in rows 32rho..32rho+32 at
        # every 32-col block: out = Rep[rho].T @ T replicates quadrant rho.
        Rep = []
        for rho in range(4):
            R = pp.tile([128, 128], F32, name=f"rep{rho}", tag=f"rep{rho}")
            G.memset(R[:], 0.0)
            G.memset(R[32 * rho:32 * rho + 32, :], 1.0)
            for q in range(4):
                G.affine_select(out=R[32 * rho:32 * rho + 32, 32 * q:32 * q + 32],
                                in_=R[32 * rho:32 * rho + 32, 32 * q:32 * q + 32],
                                pattern=[[1, 32]], compare_op=AO.is_equal,
                                fill=0.0, base=0, channel_multiplier=-1)
            Rep.append(R)

        # ---- gpsimd one-time index tiles ----
        fi = pp.tile([128, 1], I32)
        G.iota(fi[:], pattern=[[1, 1]], base=0, channel_multiplier=1)
        ff = pp.tile([128, 1], F32)
        V.tensor_copy(out=ff[:], in_=fi[:])
        s1 = pp.tile([128, 1], F32)
        s2 = pp.tile([128, 1], F32)
        i1 = pp.tile([128, 1], I32)
        V.tensor_scalar(out=s1[:], in0=ff[:], scalar1=1.0 / 32, scalar2=None, op0=AO.mult)
        V.tensor_copy(out=i1[:], in_=s1[:])
        V.tensor_copy(out=s2[:], in_=i1[:])          # rho = p//32
        s3 = pp.tile([128, 1], F32)
        V.tensor_tensor(out=s3[:], in0=s2[:], in1=s1[:], op=AO.is_gt)
        V.tensor_tensor(out=s2[:], in0=s2[:], in1=s3[:], op=AO.subtract)
        fmod = pp.tile([128, 1], F32)
        V.scalar_tensor_tensor(out=fmod[:], in0=s2[:], scalar=-32.0, in1=ff[:],
                               op0=AO.mult, op1=AO.add)   # f = p%32
        sc0 = pp.tile([128, 1], F32)
        V.tensor_scalar(out=sc0[:], in0=fmod[:], scalar1=-1.0, scalar2=-3.0,
                        op0=AO.mult, op1=AO.add)          # -3 - f
        rho_f = s2
        rho1_f = pp.tile([128, 1], F32)                   # (rho+1)%4
        V.tensor_scalar(out=rho1_f[:], in0=rho_f[:], scalar1=1.0, scalar2=None, op0=AO.add)
        w4t = pp.tile([128, 1], F32)
        V.tensor_scalar(out=w4t[:], in0=rho1_f[:], scalar1=4.0, scalar2=None, op0=AO.is_ge)
        V.scalar_tensor_tensor(out=rho1_f[:], in0=w4t[:], scalar=-4.0, in1=rho1_f[:],
                               op0=AO.mult, op1=AO.add)

        itc = pp.tile([128, NCH], I32)
        G.iota(itc[:], pattern=[[0, NCH // 4], [1, 4]], base=3,
               channel_multiplier=0)
        tcol3 = pp.tile([128, NCH], F32)                  # c%4 + 3
        V.tensor_copy(out=tcol3[:], in_=itc[:])
        itc2 = pp.tile([128, NCH], I32)
        G.iota(itc2[:], pattern=[[0, NCH // GS], [32, GS]], base=31 - 20000,
               channel_multiplier=0)
        S_c = pp.tile([128, NCH], F32)                    # 32*(c%GS) + 31 - 20000
        V.tensor_copy(out=S_c[:], in_=itc2[:])
        # EOFF: moves t=1 chunks' source-(c-1) taps (all valid taps at rho==0,
        # c%4==1) into the per-group extra scatter region: u16 offset from the
        # main base 32*(c%GS): s=1 -> 256-32 = +224, s=5 -> 256+32-160 = +128.
        itc3 = pp.tile([128, NCH], I32)
        G.iota(itc3[:], pattern=[[0, NCH // GS], [1, GS]], base=0,
               channel_multiplier=0)
        c8f = pp.tile([128, NCH], F32)
        V.tensor_copy(out=c8f[:], in_=itc3[:])
        e1 = pp.tile([128, NCH], F32)
        e5 = pp.tile([128, NCH], F32)
        V.tensor_scalar(out=e1[:], in0=c8f[:], scalar1=1.0, scalar2=224.0,
                        op0=AO.is_equal, op1=AO.mult)
        V.tensor_scalar(out=e5[:], in0=c8f[:], scalar1=5.0, scalar2=128.0,
                        op0=AO.is_equal, op1=AO.mult)
        EOFF = pp.tile([128, NCH], F32)
        V.memset(EOFF[:], 0.0)
        V.tensor_tensor(out=EOFF[0:32, :], in0=e1[0:32, :], in1=e5[0:32, :],
                        op=AO.add)

        # ------- natural [NP,128] -> replicated S-plane [128,NCH] -------
        NQ = 16  # (b, quantity) pairs: quantity 0 = zf, 1..7 = blk[j]
        T_all = pp.tile([128, NQ * 128], F32)

        zfR = [pp.tile([128, NCH], F32, name=f"zfR{b}", tag=f"zfR{b}")
               for b in range(B_LOC)]
        blkR = [pp.tile([128, NCH, 7], BF16, name=f"blkR{b}", tag=f"blkR{b}")
                for b in range(B_LOC)]
        xI = pp.tile([128, 2 * NP], F32)   # transposed x, col 2*P+... seq-interleaved

        def nat_transpose(src_ap, k, engine):
            """[NP,128] natural block -> T_all[:, 128k:128k+128]."""
            ps = psum_t.tile([128, 128], F32, name="ps_t", tag="ps_t")
            nc.tensor.transpose(ps[:], src_ap, ident[:])
            if engine == 0:
                V.tensor_copy(out=T_all[:, 128 * k:128 * (k + 1)], in_=ps[:])
            else:
                SC.activation(out=T_all[:, 128 * k:128 * (k + 1)], in_=ps[:],
                              func=AF.Copy, bias=0.0, scale=1.0)

        def rep_evac(kk, rho, psR, off, engine):
            """psR[:, 128*off:...] -> strided S-plane destination for pair kk."""
            b, q = kk % 2, kk // 2
            if q == 0:
                dst = zfR[b][:].rearrange("p (P r) -> p P r", r=4)[:, :, rho]
            else:
                dst = (blkR[b][:].rearrange("p (P r) j -> p P r j", r=4)
                       [:, :, rho, q - 1])
            src = psR[:, 128 * off:128 * (off + 1)]
            if engine == 0:
                V.tensor_copy(out=dst, in_=src)
            else:
                SC.activation(out=dst, in_=src, func=AF.Copy, bias=0.0, scale=1.0)

        # zf transposes first (pairs k=0,1), then replicate -> zfR early
        for b in range(B_LOC):
            nat_transpose(zf[:, b * 128:(b + 1) * 128], b, engine=1)
        for rho in range(4):
            psR = psum_r.tile([128, 256], F32, name="ps_rz", tag="ps_t")
            nc.tensor.matmul(psR[:], Rep[rho][:], T_all[:, 0:256],
                             start=True, stop=True)
            for kk in range(2):
                rep_evac(kk, rho, psR, kk, engine=kk % 2)

        # remaining tap math on DVE while zf replication + index math proceed
        D = f0c
        V.tensor_tensor(out=D[:], in0=f0c[:], in1=zf[:], op=AO.subtract)
        u = [pp.tile([NP, 256], F32, name=f"u{m}", tag=f"u{m}") for m in range(6)]
        for m in range(6):
            V.tensor_scalar(out=u[m][:], in0=D[:], scalar1=float(2 - m),
                            scalar2=None, op0=AO.add)
        pre = [None] * 6
        suf = [None] * 7
        pre[1] = u[0]
        for j in range(2, 6):
            pre[j] = pp.tile([NP, 256], F32, name=f"pre{j}", tag=f"pre{j}")
            V.tensor_tensor(out=pre[j][:], in0=pre[j - 1][:], in1=u[j - 1][:], op=AO.mult)
        suf[5] = u[5]
        for j in range(4, 0, -1):
            suf[j] = pp.tile([NP, 256], F32, name=f"suf{j}", tag=f"suf{j}")
            V.tensor_tensor(out=suf[j][:], in0=suf[j + 1][:], in1=u[j][:], op=AO.mult)
        w = [pp.tile([NP, 256], F32, name=f"w{j}", tag=f"w{j}") for j in range(6)]
        V.tensor_scalar(out=w[0][:], in0=suf[1][:], scalar1=INV_D[0], scalar2=None, op0=AO.mult)
        for j in range(1, 5):
            V.scalar_tensor_tensor(out=w[j][:], in0=pre[j][:], scalar=INV_D[j],
                                   in1=suf[j + 1][:], op0=AO.mult, op1=AO.mult)
        V.tensor_scalar(out=w[5][:], in0=pre[5][:], scalar1=INV_D[5], scalar2=None, op0=AO.mult)

        blk = [pp.tile([NP, 256], F32, name=f"blk{j}", tag=f"blk{j}") for j in range(7)]
        V.tensor_tensor(out=blk[0][:], in0=b0t[:], in1=w[0][:], op=AO.mult)
        for j in range(1, 6):
            V.tensor_tensor(out=blk[j][:], in0=b0t[:], in1=w[j][:], op=AO.mult)
            V.tensor_tensor(out=tmp1[:], in0=t_gp[:], in1=w[j - 1][:], op=AO.mult)
            V.tensor_tensor(out=blk[j][:], in0=blk[j][:], in1=tmp1[:], op=AO.add)
        V.tensor_tensor(out=blk[6][:], in0=t_gp[:], in1=w[5][:], op=AO.mult)

        # blk + x transposes
        for j in range(7):
            for b in range(B_LOC):
                nat_transpose(blk[j][:, b * 128:(b + 1) * 128], 2 * (j + 1) + b,
                              engine=1)
        for b in range(B_LOC):
            ps = psum_t.tile([128, 128], F32, name="ps_x", tag="ps_t")
            nc.tensor.transpose(ps[:], nat_x[:, b * 128:(b + 1) * 128], ident[:])
            V.tensor_copy(out=xI[:].rearrange("p (P s) -> p P s", s=2)[:, :, b],
                          in_=ps[:])
        # replicate blk planes: T_all cols 256..2048
        for rho in range(4):
            for col0, npair in ((256, 4), (768, 4), (1280, 4), (1792, 2)):
                psR = psum_r.tile([128, 128 * npair], F32, name="ps_rb", tag="ps_t")
                nc.tensor.matmul(psR[:], Rep[rho][:],
                                 T_all[:, col0:col0 + 128 * npair],
                                 start=True, stop=True)
                for kk in range(npair):
                    rep_evac(col0 // 128 + kk, rho, psR, kk,
                             engine=kk % 2)

        # ---------------- scatter index computation (GPSIMD) ----------------
        # v0 = zf - 3 - f; fl = v0//32; m = v0%32; uB = c%4 + 3 - fl
        # no-wrap valid: (uB%4 == rho); wrap valid: (uB%4 == (rho+1)%4)
        # idx[j] = valid_j*20000 + 32*iw_j + 31 - m - j - 20000 + 32*(c%GS)
        idxR = [pp.tile([128, NCH, 7], I16, name=f"idxR{b}", tag=f"idxR{b}")
                for b in range(B_LOC)]
        gv0 = pp.tile([128, NCH], F32)
        gtA = pp.tile([128, NCH], F32)
        gfl_i = pp.tile([128, NCH], I16)
        gflf = pp.tile([128, NCH], F32)
        gm0 = pp.tile([128, NCH], F32)
        guB = pp.tile([128, NCH], F32)
        gw4 = pp.tile([128, NCH], F32)
        givA = pp.tile([128, NCH], F32)
        givB = pp.tile([128, NCH], F32)
        gbase = pp.tile([128, NCH], F32)
        gdv = pp.tile([128, NCH], F32)
        gdiff = pp.tile([128, NCH], F32)
        gidxA = pp.tile([128, NCH], F32)
        giw = pp.tile([128, NCH], F32)
        gt = pp.tile([128, NCH], F32)
        gtj = pp.tile([128, NCH], F32)

        NSL = 4                       # column slices for early scatter start
        SW = NCH // NSL

        def idx_math(b, sl, sink=None):
            def emit(f, j=None):
                if sink is None:
                    f(j)
                else:
                    sink.append(lambda jj=j: f(jj))
            cs = slice(sl * SW, (sl + 1) * SW)
            emit(lambda _j: (
                V.tensor_scalar(out=gv0[:, cs], in0=zfR[b][:, cs], scalar1=sc0[:],
                                scalar2=None, op0=AO.add)
            ))
            emit(lambda _j: (
                V.tensor_scalar(out=gtA[:, cs], in0=gv0[:, cs], scalar1=1.0 / 32,
                                scalar2=None, op0=AO.mult)
            ))
            emit(lambda _j: (
                V.tensor_copy(out=gfl_i[:, cs], in_=gtA[:, cs])
            ))
            emit(lambda _j: (
                V.tensor_copy(out=gflf[:, cs], in_=gfl_i[:, cs])
            ))
            emit(lambda _j: (
                V.tensor_tensor(out=gw4[:, cs], in0=gflf[:, cs], in1=gtA[:, cs],
                                op=AO.is_gt)
            ))
            emit(lambda _j: (
                V.tensor_tensor(out=gflf[:, cs], in0=gflf[:, cs], in1=gw4[:, cs],
                                op=AO.subtract)
            ))
            emit(lambda _j: (
                V.scalar_tensor_tensor(out=gm0[:, cs], in0=gflf[:, cs], scalar=-32.0,
                                       in1=gv0[:, cs], op0=AO.mult, op1=AO.add)
            ))
            emit(lambda _j: (
                V.scalar_tensor_tensor(out=guB[:, cs], in0=gflf[:, cs], scalar=-1.0,
                                       in1=tcol3[:, cs], op0=AO.mult, op1=AO.add)
            ))
            emit(lambda _j: (
                V.tensor_scalar(out=gw4[:, cs], in0=guB[:, cs], scalar1=4.0,
                                scalar2=None, op0=AO.is_ge)
            ))
            emit(lambda _j: (
                V.scalar_tensor_tensor(out=guB[:, cs], in0=gw4[:, cs], scalar=-4.0,
                                       in1=guB[:, cs], op0=AO.mult, op1=AO.add)
            ))
            emit(lambda _j: (
                V.tensor_scalar(out=givA[:, cs], in0=guB[:, cs], scalar1=rho_f[:],
                                scalar2=None, op0=AO.is_equal)
            ))
            emit(lambda _j: (
                V.tensor_scalar(out=givB[:, cs], in0=guB[:, cs], scalar1=rho1_f[:],
                                scalar2=None, op0=AO.is_equal)
            ))
            emit(lambda _j: (
                V.scalar_tensor_tensor(out=gbase[:, cs], in0=gm0[:, cs], scalar=-1.0,
                                       in1=S_c[:, cs], op0=AO.mult, op1=AO.add)
            ))
            emit(lambda _j: (
                V.tensor_tensor(out=gbase[:, cs], in0=gbase[:, cs],
                                in1=EOFF[:, cs], op=AO.add)
            ))
            emit(lambda _j: (
                V.tensor_tensor(out=gdv[:, cs], in0=givB[:, cs], in1=givA[:, cs],
                                op=AO.subtract)
            ))
            emit(lambda _j: (
                V.tensor_scalar(out=gdiff[:, cs], in0=gdv[:, cs], scalar1=20000.0,
                                scalar2=32.0, op0=AO.mult, op1=AO.add)
            ))
            emit(lambda _j: (
                V.scalar_tensor_tensor(out=gidxA[:, cs], in0=givA[:, cs], scalar=20000.0,
                                       in1=gbase[:, cs], op0=AO.mult, op1=AO.add)
            ))
            for j in range(7):
                emit(lambda j: (
                    V.tensor_scalar(out=giw[:, cs], in0=gm0[:, cs],
                                    scalar1=float(32 - j), scalar2=None, op0=AO.is_ge)
                ), j)
                emit(lambda _j: (
                    V.tensor_tensor(out=gt[:, cs], in0=giw[:, cs], in1=gdiff[:, cs],
                                    op=AO.mult)
                ))
                emit(lambda j: (
                    V.scalar_tensor_tensor(out=idxR[b][:, cs, j], in0=gt[:, cs],
                                           scalar=float(-j), in1=gidxA[:, cs],
                                           op0=AO.add, op1=AO.add)
                ), j)

        # -------------- interleaved ring + chain ----------------
        NRC = NCH // 4 + 1
        ringI = pp.tile([128, 2 * NRC], BF16)
        V.memset(ringI[:], 0.0)

        blkR_u16 = [blkR[b][:].bitcast(U16) for b in range(B_LOC)]

        traws = {}
        t4s_map = {}

        def scatter_group(g):
            pair = []
            for b in range(B_LOC):
                traw = traw_pool.tile([128, 32 * (GS + 2)], BF16, name="traw",
                                      tag=f"traw{b}")
                G.local_scatter(
                    out_ap=traw[:].bitcast(U16),
                    data_ap=blkR_u16[b][:, GS * g:GS * (g + 1), :]
                    .rearrange("p c j -> p (c j)"),
                    idxs_ap=idxR[b][:, GS * g:GS * (g + 1), :]
                    .rearrange("p c j -> p (c j)"),
                    channels=128, num_elems=32 * (GS + 2), num_idxs=7 * GS,
                )
                pair.append(traw)
            traws[g] = pair
            t4s_map[g] = [t4_pool.tile([128, 32 * (GS + 2)], BF16, name="t4",
                                       tag=f"t4{b}") for b in range(B_LOC)]

        HW = 32 * (GS + 2) // 2  # transpose half-width (multiple of 32)

        def transpose_half(g, b, h):
            cs = slice(h * HW, (h + 1) * HW)
            nc.vector.transpose(out=t4s_map[g][b][:, cs], in_=traws[g][b][:, cs])

        for b in range(B_LOC):
            idx_math(b, 0)
        pending = []
        for sl in range(1, NSL):
            for b in range(B_LOC):
                idx_math(b, sl, sink=pending)
        pending.reverse()

        for g0 in range(2):
            scatter_group(g0)
            for b in range(B_LOC):
                for h in range(2):
                    transpose_half(g0, b, h)

        # incremental output transpose: segment k (ynat rows 32k..32k+32,
        # i.e. ring groups 32k..32k+32) is transposed+copied as soon as its
        # ring columns are final, so only the last segment remains at the end
        ynat = [pp.tile([NP, 128], F32, name=f"ynat{b}", tag=f"ynat{b}")
                for b in range(B_LOC)]

        def emit_out_segment(k, b, g0):
            src = (ringI[32 * g0:32 * g0 + 32, 2 + 64 * k:2 + 64 * k + 64]
                   .rearrange("p (P s) -> p P s", s=2)[:, :, b])
            ps = psum_t.tile([128, 32], BF16, name="ps_o", tag="ps_t")
            nc.tensor.transpose(ps[32 * k:32 * k + 32, :], src,
                                ident4b[32 * g0:32 * g0 + 32, :],
                                tile_position=(32 * g0, 32 * k))
            dst = ynat[b][32 * k:32 * k + 32, 32 * g0:32 * g0 + 32]
            if b == 0:
                V.tensor_copy(out=dst, in_=ps[32 * k:32 * k + 32, :])
            else:
                SC.activation(out=dst, in_=ps[32 * k:32 * k + 32, :],
                              func=AF.Copy, bias=0.0, scale=1.0)

        def emit_out_dma(k, b):
            nc.sync.dma_start(
                out=out_d[b].rearrange("(P j) -> P j", j=128)[32 * k:32 * k + 32, :],
                in_=ynat[b][32 * k:32 * k + 32, :],
            )

        for g in range(NG):
            t4s = t4s_map.pop(g)
            for s in range(GS):
                c = GS * g + s
                gg, t = c // 4, c % 4
                for b in range(B_LOC):
                    ps = psum_c.tile([128, 1], F32, name=f"ps{b}", tag=f"ps{b}")
                    pieces = PIECES[t]
                    for i, (rb, rs, cd, cb) in enumerate(pieces):
                        rows = slice(rb, rb + rs)
                        nu = gg + cd
                        col0 = (32 * GS + 32 * ((s - 1) // 4)
                                if cb == "X" else 32 * s)
                        nc.tensor.matmul(
                            ps[32 * t:32 * t + 32, 0:1],
                            t4s[b][rows, col0:col0 + 32],
                            ringI[rows, 2 * nu + b:2 * nu + b + 1],
                            start=(i == 0), stop=(i == len(pieces) - 1),
                            tile_position=(rb, 32 * t),
                        )
                    ring_dst = ringI[32 * t:32 * t + 32,
                                     2 * (1 + gg) + b:2 * (1 + gg) + b + 1]
                    x_src = xI[32 * t:32 * t + 32, 2 * gg + b:2 * gg + b + 1]
                    if b == 0:
                        V.scalar_tensor_tensor(
                            out=ring_dst, in0=ps[32 * t:32 * t + 32, 0:1],
                            scalar=1.0, in1=x_src, op0=AO.mult, op1=AO.add,
                        )
                    else:
                        SC.add(out=ring_dst, in_=ps[32 * t:32 * t + 32, 0:1],
                               add=x_src)
                for _ in range(2):
                    if pending:
                        pending.pop()()
                gn = g + 2
                if gn < NG:
                    if s == 1:
                        scatter_group(gn)
                    elif s in (3, 4, 5, 6):
                        bb, hh = divmod(s - 3, 2)
                        transpose_half(gn, bb, hh)
                if g % 16 == 0 and g >= 16:
                    emit_out_segment(g // 16 - 1, s % 2, s // 2)
                elif g % 16 == 1 and g >= 16 and s < 2:
                    emit_out_dma(g // 16 - 1, s)

        # ---------------- final output segment + store ----------------
        for b in range(B_LOC):
            for g0 in range(4):
                emit_out_segment(3, b, g0)
        for b in range(B_LOC):
            emit_out_dma(3, b)


def build_program(N=N_FULL):
    nc = bacc.Bacc("TRN2", target_bir_lowering=False, debug=False,
                   enable_asserts=False)
    f0_d = nc.dram_tensor("f0", [B_LOC, N], F32, kind="ExternalInput").ap()
    x_d = nc.dram_tensor("x", [B_LOC, N], F32, kind="ExternalInput").ap()
    lb_d = nc.dram_tensor("l_b", [B_LOC, N, 2], F32, kind="ExternalInput").ap()
    out_d = nc.dram_tensor("out", [B_LOC, N], F32, kind="ExternalOutput").ap()
    with tile.TileContext(nc) as tc:
        build_kernel(tc, out_d, f0_d, x_d, lb_d, N)
    nc.compile()
    return nc


_PROGRAM_CACHE = {}


def _get_program(N=N_FULL):
    if N not in _PROGRAM_CACHE:
        _PROGRAM_CACHE[N] = build_program(N)
    return _PROGRAM_CACHE[N]


def kernel(f0, x, l_b, K=108, **kwargs):
    """Full-input entry point: shards batch across 8 cores, returns full output."""
    f0 = np.asarray(f0, dtype=np.float32)
    x = np.asarray(x, dtype=np.float32)
    l_b = np.asarray(l_b, dtype=np.float32)
    B, N = x.shape
    assert B == B_FULL and int(K) == 108
    nc = _get_program(N)
    in_maps = []
    for i in range(NCORES):
        sl = slice(i * B_LOC, (i + 1) * B_LOC)
        in_maps.append({
            "f0": np.ascontiguousarray(f0[sl]),
            "x": np.ascontiguousarray(x[sl]),
            "l_b": np.ascontiguousarray(l_b[sl]),
        })
    res = bass_utils.run_bass_kernel_spmd(nc, in_maps, core_ids=list(range(NCORES)))
    out = np.concatenate([res.results[i]["out"] for i in range(NCORES)], axis=0)
    return out.astype(np.float32)



# revision 35
# speedup vs baseline: 1.0739x; 1.0739x over previous
"""Trainium2 Bass kernel for DiffKS (differentiable Karplus-Strong string).

Math (per sequence b, time n):
    g = 0.99*l_b[...,0]; p = l_b[...,1]
    b0 = g*(1-p); a1 = g*p
    f0c = f0 - a1/(b0+a1+1e-7)
    z = floor(f0c); zc = z-2; alpha = f0c - zc
    w_j = Lagrange weights (order 5), j=0..5
    block_j = b0*w_j + a1*w_{j-1}, j=0..6           (7 taps)
    taps live at k = c0+j, c0 = zc-1 = z-3 in [36, 96]
    y[n] = x[n] + sum_j block_j[n] * y[n-1-(c0[n]+j)]    (delays 37..103)

Key structure: minimum delay is 37 > 32, so 32-sample chunks are internally
parallel.  Chunk c is computed as accumulating PE matmuls against the previous
4 chunks' outputs, with per-chunk tap matrices built on-chip by a GPSIMD
local_scatter + DVE 32x32 block transpose.  B=16 is sharded 2 seqs/core.

Phase-1 optimized layout vs the original baseline:
  - both sequences share one interleaved ring tile ringI[128, 2*(NCH/4+1)]
    (col 2*nu+b) and one psum tile [128, 2] per chunk, so each chunk needs a
    single [32,2] DVE evac instead of two.
  - matmul pieces with contiguous rows are merged (avg 1.75 vs 2.25 per
    chunk per seq).
  - natural->S-plane transposes are done as 16 full 128x128 PE transposes
    plus 4-replication matmuls with shared stationary (Rep_rho), evacuated
    with strided copies split between DVE and ACT.
  - scatter index math is reduced (~42 ops/seq) and runs on GPSIMD,
    overlapped with the tap math / transposes, sliced so scatters start
    before all index math finishes.

Layouts (per core, seqs b=0,1; chunk T=32; NCH = N/32 chunks; NP = N/128):
  natural plane  nat[P, b*128+j]  = q[b, 128*P + j]          [NP, 256]
  S-plane        qS[32*rho+f, c]  = q[b, 32*c + f], c = 4P+rho (replicated
                 over rho for scatter source planes)          [128, NCH]
  ring           ringI[32*(c%4)+f, 2*(1+c//4)+b] = y[b, 32*c+f]
Tap matrix for chunk c (lhsT for the PE matmul): rows 32*fl + (31 - m)
address the ring window column; scatter writes single u16s of bf16 taps.

Phase-2 (this session): chain data in bf16 (taps + ring; psum accumulation
stays fp32) -> single-pass PE matmuls instead of fp32 LOW/HIGH, half the
scatter indices, 2x faster DVE transposes; chain evacs split DVE (seq 0) /
ACT (seq 1) so the two evacs run concurrently and ACT is off the DVE queue.
Verified offline: bf16 taps+ring gives ~2e-3 rel err (budget 2e-2).
"""

import numpy as np

import concourse.bass as bass
import concourse.mybir as mybir
import concourse.bacc as bacc
import concourse.tile as tile
from concourse import bass_utils

F32 = mybir.dt.float32
BF16 = mybir.dt.bfloat16
I32 = mybir.dt.int32
I16 = mybir.dt.int16
U16 = mybir.dt.uint16
AO = mybir.AluOpType
AF = mybir.ActivationFunctionType

B_FULL = 16
N_FULL = 16384
NCORES = 8
B_LOC = 2  # sequences per core
GS = 8     # chunks per scatter group

# matmul piece tables per t=c%4: (row_base, row_size, col_delta); ring column
# read is (c//4) + col_delta.  Contiguous same-col-delta rows are merged where
# tile_position allows (row base 0 for sizes > 64); the tile's row space is
# shared between col deltas, so pieces must never overlap rows.
# col base None = main region (32*s); "X" = extra region for t=1's c-1 piece
PIECES = {
    0: [(0, 128, 0, None)],
    1: [(0, 128, 0, None), (0, 32, 1, "X")],
    2: [(64, 64, 0, None), (0, 64, 1, None)],
    3: [(96, 32, 0, None), (0, 96, 1, None)],
}

# Lagrange denominators 1/d_j for order 5
INV_D = [-1.0 / 120, 1.0 / 24, -1.0 / 12, 1.0 / 12, -1.0 / 24, 1.0 / 120]


def build_kernel(tc, out_d, f0_d, x_d, lb_d, N):
    nc = tc.nc
    NP = N // 128          # natural-plane columns per seq
    NCH = N // 32          # chunks per seq
    NG = NCH // GS         # scatter groups
    assert NP * 128 == N and NP == 128 and NG * GS == NCH

    import contextlib
    ctx = contextlib.ExitStack()
    pp = ctx.enter_context(tc.tile_pool(name="persist", bufs=1))
    traw_pool = ctx.enter_context(tc.tile_pool(name="traw", bufs=4))
    t4_pool = ctx.enter_context(tc.tile_pool(name="t4", bufs=12))
    psum_t = ctx.enter_context(tc.tile_pool(name="psum_t", bufs=2, space="PSUM"))
    psum_r = psum_t
    psum_c = ctx.enter_context(tc.tile_pool(name="psum_c", bufs=3, space="PSUM"))

    V = nc.vector
    G = nc.gpsimd
    SC = nc.scalar

    with ctx:
        # ---------------- phase 0: load + elementwise tap math ----------------
        nat_f0 = pp.tile([NP, 256], F32)
        nat_x = pp.tile([NP, 256], F32)
        nat_lb = pp.tile([NP, 512], F32)
        for b in range(B_LOC):
            nc.sync.dma_start(
                out=nat_f0[:, b * 128:(b + 1) * 128],
                in_=f0_d[b].rearrange("(p j) -> p j", j=128),
            )
            nc.sync.dma_start(
                out=nat_x[:, b * 128:(b + 1) * 128],
                in_=x_d[b].rearrange("(p j) -> p j", j=128),
            )
            nc.sync.dma_start(
                out=nat_lb[:, b * 256:(b + 1) * 256],
                in_=lb_d[b].rearrange("(p j) s -> p (j s)", j=128),
            )
        lb_r = nat_lb[:].rearrange("p (j s) -> p j s", s=2)
        g_ap = lb_r[:, :, 0]
        p_ap = lb_r[:, :, 1]

        g99 = pp.tile([NP, 256], F32)
        t_gp = pp.tile([NP, 256], F32)   # a1 = 0.99*g*p
        b0t = pp.tile([NP, 256], F32)
        rec = pp.tile([NP, 256], F32)
        f0c = pp.tile([NP, 256], F32)
        zf = pp.tile([NP, 256], F32)
        tmp1 = pp.tile([NP, 256], F32)
        tmp2 = pp.tile([NP, 256], F32)
        itmp = pp.tile([NP, 256], I32)

        # zf chain first so its transposes (and gpsimd index math) start early
        V.tensor_scalar(out=g99[:], in0=g_ap, scalar1=0.99, scalar2=None, op0=AO.mult)
        V.tensor_tensor(out=t_gp[:], in0=g99[:], in1=p_ap, op=AO.mult)       # a1
        V.tensor_tensor(out=b0t[:], in0=g99[:], in1=t_gp[:], op=AO.subtract)  # b0
        V.tensor_scalar(out=tmp1[:], in0=g99[:], scalar1=1e-7, scalar2=None, op0=AO.add)
        V.reciprocal(out=rec[:], in_=tmp1[:])
        V.tensor_tensor(out=tmp2[:], in0=t_gp[:], in1=rec[:], op=AO.mult)
        V.tensor_tensor(out=f0c[:], in0=nat_f0[:], in1=tmp2[:], op=AO.subtract)
        # zf = floor(f0c), robust to cast rounding mode
        V.tensor_copy(out=itmp[:], in_=f0c[:])
        V.tensor_copy(out=zf[:], in_=itmp[:])
        V.tensor_tensor(out=tmp1[:], in0=zf[:], in1=f0c[:], op=AO.is_gt)
        V.tensor_tensor(out=zf[:], in0=zf[:], in1=tmp1[:], op=AO.subtract)

        # ---- transpose machinery (identity + replication stationaries) ----
        ident = pp.tile([128, 128], F32)
        G.memset(ident[:], 1.0)
        G.affine_select(out=ident[:], in_=ident[:], pattern=[[1, 128]],
                        compare_op=AO.is_equal, fill=0.0, base=0,
                        channel_multiplier=-1)
        identb = pp.tile([128, 128], BF16)
        G.memset(identb[:], 1.0)
        G.affine_select(out=identb[:], in_=identb[:], pattern=[[1, 128]],
                        compare_op=AO.is_equal, fill=0.0, base=0,
                        channel_multiplier=-1)
        ident4b = pp.tile([128, 32], BF16)
        G.memset(ident4b[:], 1.0)
        for g0 in range(4):
            G.affine_select(out=ident4b[32 * g0:32 * g0 + 32, :],
                            in_=ident4b[32 * g0:32 * g0 + 32, :],
                            pattern=[[1, 32]], compare_op=AO.is_equal,
                            fill=0.0, base=0, channel_multiplier=-1)
        # Rep[rho]: [128,128] with identity blocks in rows 32rho..32rho+32 at
        # every 32-col block: out = Rep[rho].T @ T replicates quadrant rho.
        Rep = []
        Repb = []
        for rho in range(4):
            for lst, dt, nm in ((Rep, F32, "rep"), (Repb, BF16, "repb")):
                R = pp.tile([128, 128], dt, name=f"{nm}{rho}", tag=f"{nm}{rho}")
                G.memset(R[:], 0.0)
                G.memset(R[32 * rho:32 * rho + 32, :], 1.0)
                for q in range(4):
                    G.affine_select(
                        out=R[32 * rho:32 * rho + 32, 32 * q:32 * q + 32],
                        in_=R[32 * rho:32 * rho + 32, 32 * q:32 * q + 32],
                        pattern=[[1, 32]], compare_op=AO.is_equal,
                        fill=0.0, base=0, channel_multiplier=-1)
                lst.append(R)

        # ---- gpsimd one-time index tiles ----
        fi = pp.tile([128, 1], I32)
        G.iota(fi[:], pattern=[[1, 1]], base=0, channel_multiplier=1)
        ff = pp.tile([128, 1], F32)
        V.tensor_copy(out=ff[:], in_=fi[:])
        s1 = pp.tile([128, 1], F32)
        s2 = pp.tile([128, 1], F32)
        i1 = pp.tile([128, 1], I32)
        V.tensor_scalar(out=s1[:], in0=ff[:], scalar1=1.0 / 32, scalar2=None, op0=AO.mult)
        V.tensor_copy(out=i1[:], in_=s1[:])
        V.tensor_copy(out=s2[:], in_=i1[:])          # rho = p//32
        s3 = pp.tile([128, 1], F32)
        V.tensor_tensor(out=s3[:], in0=s2[:], in1=s1[:], op=AO.is_gt)
        V.tensor_tensor(out=s2[:], in0=s2[:], in1=s3[:], op=AO.subtract)
        fmod = pp.tile([128, 1], F32)
        V.scalar_tensor_tensor(out=fmod[:], in0=s2[:], scalar=-32.0, in1=ff[:],
                               op0=AO.mult, op1=AO.add)   # f = p%32
        sc0 = pp.tile([128, 1], F32)
        V.tensor_scalar(out=sc0[:], in0=fmod[:], scalar1=-1.0, scalar2=-3.0,
                        op0=AO.mult, op1=AO.add)          # -3 - f
        rho_f = s2
        rho1_f = pp.tile([128, 1], F32)                   # (rho+1)%4
        V.tensor_scalar(out=rho1_f[:], in0=rho_f[:], scalar1=1.0, scalar2=None, op0=AO.add)
        w4t = pp.tile([128, 1], F32)
        V.tensor_scalar(out=w4t[:], in0=rho1_f[:], scalar1=4.0, scalar2=None, op0=AO.is_ge)
        V.scalar_tensor_tensor(out=rho1_f[:], in0=w4t[:], scalar=-4.0, in1=rho1_f[:],
                               op0=AO.mult, op1=AO.add)

        itc = pp.tile([128, NCH], I32)
        G.iota(itc[:], pattern=[[0, NCH // 4], [1, 4]], base=3,
               channel_multiplier=0)
        tcol3 = pp.tile([128, NCH], F32)                  # c%4 + 3
        V.tensor_copy(out=tcol3[:], in_=itc[:])
        itc2 = pp.tile([128, NCH], I32)
        G.iota(itc2[:], pattern=[[0, NCH // GS], [32, GS]], base=31 - 20000,
               channel_multiplier=0)
        S_c = pp.tile([128, NCH], F32)                    # 32*(c%GS) + 31 - 20000
        V.tensor_copy(out=S_c[:], in_=itc2[:])
        # EOFF: moves t=1 chunks' source-(c-1) taps (all valid taps at rho==0,
        # c%4==1) into the per-group extra scatter region: u16 offset from the
        # main base 32*(c%GS): s=1 -> 256-32 = +224, s=5 -> 256+32-160 = +128.
        itc3 = pp.tile([128, NCH], I32)
        G.iota(itc3[:], pattern=[[0, NCH // GS], [1, GS]], base=0,
               channel_multiplier=0)
        c8f = pp.tile([128, NCH], F32)
        V.tensor_copy(out=c8f[:], in_=itc3[:])
        e1 = pp.tile([128, NCH], F32)
        e5 = pp.tile([128, NCH], F32)
        V.tensor_scalar(out=e1[:], in0=c8f[:], scalar1=1.0, scalar2=224.0,
                        op0=AO.is_equal, op1=AO.mult)
        V.tensor_scalar(out=e5[:], in0=c8f[:], scalar1=5.0, scalar2=128.0,
                        op0=AO.is_equal, op1=AO.mult)
        EOFF = pp.tile([128, NCH], F32)
        V.memset(EOFF[:], 0.0)
        V.tensor_tensor(out=EOFF[0:32, :], in0=e1[0:32, :], in1=e5[0:32, :],
                        op=AO.add)

        # ------- natural [NP,128] -> replicated S-plane [128,NCH] -------
        # pairs: 0..1 = zf (fp32); T_blk holds the 14 blk pairs in bf16
        T_all = pp.tile([128, 2 * 128], F32)
        T_blk = pp.tile([128, 14 * 128], BF16)

        zfR = [pp.tile([128, NCH], F32, name=f"zfR{b}", tag=f"zfR{b}")
               for b in range(B_LOC)]
        blkR = [pp.tile([128, NCH, 7], BF16, name=f"blkR{b}", tag=f"blkR{b}")
                for b in range(B_LOC)]
        xI = pp.tile([128, 2 * NP], F32)   # transposed x, col 2*P+... seq-interleaved

        def nat_transpose(src_ap, k, engine, bf=False):
            """[NP,128] natural block -> (T_blk if bf else T_all) pair k."""
            dt = BF16 if bf else F32
            ps = psum_t.tile([128, 128], dt, name="ps_t", tag="ps_t")
            nc.tensor.transpose(ps[:], src_ap, (identb if bf else ident)[:])
            dst_t = T_blk if bf else T_all
            dst = dst_t[:, 128 * k:128 * (k + 1)]
            if engine == 0:
                V.tensor_copy(out=dst, in_=ps[:])
            else:
                SC.activation(out=dst, in_=ps[:],
                              func=AF.Copy, bias=0.0, scale=1.0)

        def rep_evac(kk, rho, psR, off, engine):
            """psR[:, 128*off:...] -> strided S-plane destination for pair kk."""
            b, q = kk % 2, kk // 2
            if q == 0:
                dst = zfR[b][:].rearrange("p (P r) -> p P r", r=4)[:, :, rho]
            else:
                dst = (blkR[b][:].rearrange("p (P r) j -> p P r j", r=4)
                       [:, :, rho, q - 1])
            src = psR[:, 128 * off:128 * (off + 1)]
            if engine == 0:
                V.tensor_copy(out=dst, in_=src)
            else:
                SC.activation(out=dst, in_=src, func=AF.Copy, bias=0.0, scale=1.0)

        # zf transposes first (pairs k=0,1), then replicate -> zfR early
        for b in range(B_LOC):
            nat_transpose(zf[:, b * 128:(b + 1) * 128], b, engine=1)
        for rho in range(4):
            psR = psum_r.tile([128, 256], F32, name="ps_rz", tag="ps_t")
            nc.tensor.matmul(psR[:], Rep[rho][:], T_all[:, 0:256],
                             start=True, stop=True)
            for kk in range(2):
                rep_evac(kk, rho, psR, kk, engine=1)

        # remaining tap math on DVE while zf replication + index math proceed
        D = f0c
        V.tensor_tensor(out=D[:], in0=f0c[:], in1=zf[:], op=AO.subtract)
        u = [pp.tile([NP, 256], F32, name=f"u{m}", tag=f"u{m}") for m in range(6)]
        for m in range(6):
            V.tensor_scalar(out=u[m][:], in0=D[:], scalar1=float(2 - m),
                            scalar2=None, op0=AO.add)
        pre = [None] * 6
        suf = [None] * 7
        pre[1] = u[0]
        for j in range(2, 6):
            pre[j] = pp.tile([NP, 256], F32, name=f"pre{j}", tag=f"pre{j}")
            V.tensor_tensor(out=pre[j][:], in0=pre[j - 1][:], in1=u[j - 1][:], op=AO.mult)
        suf[5] = u[5]
        for j in range(4, 0, -1):
            suf[j] = pp.tile([NP, 256], F32, name=f"suf{j}", tag=f"suf{j}")
            V.tensor_tensor(out=suf[j][:], in0=suf[j + 1][:], in1=u[j][:], op=AO.mult)
        w = [pp.tile([NP, 256], F32, name=f"w{j}", tag=f"w{j}") for j in range(6)]
        V.tensor_scalar(out=w[0][:], in0=suf[1][:], scalar1=INV_D[0], scalar2=None, op0=AO.mult)
        for j in range(1, 5):
            V.scalar_tensor_tensor(out=w[j][:], in0=pre[j][:], scalar=INV_D[j],
                                   in1=suf[j + 1][:], op0=AO.mult, op1=AO.mult)
        V.tensor_scalar(out=w[5][:], in0=pre[5][:], scalar1=INV_D[5], scalar2=None, op0=AO.mult)

        blk = [pp.tile([NP, 256], BF16, name=f"blk{j}", tag=f"blk{j}") for j in range(7)]
        V.tensor_tensor(out=blk[0][:], in0=b0t[:], in1=w[0][:], op=AO.mult)
        for j in range(1, 6):
            V.tensor_tensor(out=tmp2[:], in0=b0t[:], in1=w[j][:], op=AO.mult)
            V.tensor_tensor(out=tmp1[:], in0=t_gp[:], in1=w[j - 1][:], op=AO.mult)
            V.tensor_tensor(out=blk[j][:], in0=tmp2[:], in1=tmp1[:], op=AO.add)
        V.tensor_tensor(out=blk[6][:], in0=t_gp[:], in1=w[5][:], op=AO.mult)

        # blk + x transposes
        for j in range(7):
            for b in range(B_LOC):
                nat_transpose(blk[j][:, b * 128:(b + 1) * 128], 2 * j + b,
                              engine=1, bf=True)
        for b in range(B_LOC):
            ps = psum_t.tile([128, 128], F32, name="ps_x", tag="ps_t")
            nc.tensor.transpose(ps[:], nat_x[:, b * 128:(b + 1) * 128], ident[:])
            V.tensor_copy(out=xI[:].rearrange("p (P s) -> p P s", s=2)[:, :, b],
                          in_=ps[:])
        # replicate blk planes (bf16 path: 1 cycle/row on the PE)
        for rho in range(4):
            for col0, npair in ((0, 4), (512, 4), (1024, 4), (1536, 2)):
                psR = psum_r.tile([128, 128 * npair], F32, name="ps_rb", tag="ps_t")
                nc.tensor.matmul(psR[:], Repb[rho][:],
                                 T_blk[:, col0:col0 + 128 * npair],
                                 start=True, stop=True)
                for kk in range(npair):
                    rep_evac(2 + col0 // 128 + kk, rho, psR, kk,
                             engine=1)

        # ---------------- scatter index computation (GPSIMD) ----------------
        # v0 = zf - 3 - f; fl = v0//32; m = v0%32; uB = c%4 + 3 - fl
        # no-wrap valid: (uB%4 == rho); wrap valid: (uB%4 == (rho+1)%4)
        # idx[j] = valid_j*20000 + 32*iw_j + 31 - m - j - 20000 + 32*(c%GS)
        idxR = [pp.tile([128, NCH, 7], I16, name=f"idxR{b}", tag=f"idxR{b}")
                for b in range(B_LOC)]
        gv0 = pp.tile([128, NCH], F32)
        gtA = pp.tile([128, NCH], F32)
        gfl_i = pp.tile([128, NCH], I16)
        gflf = pp.tile([128, NCH], F32)
        gm0 = pp.tile([128, NCH], F32)
        guB = pp.tile([128, NCH], F32)
        gw4 = pp.tile([128, NCH], F32)
        givA = pp.tile([128, NCH], F32)
        givB = pp.tile([128, NCH], F32)
        gbase = pp.tile([128, NCH], F32)
        gdv = pp.tile([128, NCH], F32)
        gdiff = pp.tile([128, NCH], F32)
        gidxA = pp.tile([128, NCH], F32)
        giw = pp.tile([128, NCH], F32)
        gt = pp.tile([128, NCH], F32)
        gtj = pp.tile([128, NCH], F32)

        NSL = 4                       # column slices for early scatter start
        SW = NCH // NSL

        def idx_math(b, sl, sink=None):
            def emit(f, j=None):
                if sink is None:
                    f(j)
                else:
                    sink.append(lambda jj=j: f(jj))
            cs = slice(sl * SW, (sl + 1) * SW)
            emit(lambda _j: (
                V.tensor_scalar(out=gv0[:, cs], in0=zfR[b][:, cs], scalar1=sc0[:],
                                scalar2=None, op0=AO.add)
            ))
            emit(lambda _j: (
                V.tensor_scalar(out=gtA[:, cs], in0=gv0[:, cs], scalar1=1.0 / 32,
                                scalar2=None, op0=AO.mult)
            ))
            emit(lambda _j: (
                V.tensor_copy(out=gfl_i[:, cs], in_=gtA[:, cs])
            ))
            emit(lambda _j: (
                V.tensor_copy(out=gflf[:, cs], in_=gfl_i[:, cs])
            ))
            emit(lambda _j: (
                V.tensor_tensor(out=gw4[:, cs], in0=gflf[:, cs], in1=gtA[:, cs],
                                op=AO.is_gt)
            ))
            emit(lambda _j: (
                V.tensor_tensor(out=gflf[:, cs], in0=gflf[:, cs], in1=gw4[:, cs],
                                op=AO.subtract)
            ))
            emit(lambda _j: (
                V.scalar_tensor_tensor(out=gm0[:, cs], in0=gflf[:, cs], scalar=-32.0,
                                       in1=gv0[:, cs], op0=AO.mult, op1=AO.add)
            ))
            emit(lambda _j: (
                V.scalar_tensor_tensor(out=guB[:, cs], in0=gflf[:, cs], scalar=-1.0,
                                       in1=tcol3[:, cs], op0=AO.mult, op1=AO.add)
            ))
            emit(lambda _j: (
                V.tensor_scalar(out=gw4[:, cs], in0=guB[:, cs], scalar1=4.0,
                                scalar2=None, op0=AO.is_ge)
            ))
            emit(lambda _j: (
                V.scalar_tensor_tensor(out=guB[:, cs], in0=gw4[:, cs], scalar=-4.0,
                                       in1=guB[:, cs], op0=AO.mult, op1=AO.add)
            ))
            emit(lambda _j: (
                V.tensor_scalar(out=givA[:, cs], in0=guB[:, cs], scalar1=rho_f[:],
                                scalar2=None, op0=AO.is_equal)
            ))
            emit(lambda _j: (
                V.tensor_scalar(out=givB[:, cs], in0=guB[:, cs], scalar1=rho1_f[:],
                                scalar2=None, op0=AO.is_equal)
            ))
            emit(lambda _j: (
                V.scalar_tensor_tensor(out=gbase[:, cs], in0=gm0[:, cs], scalar=-1.0,
                                       in1=S_c[:, cs], op0=AO.mult, op1=AO.add)
            ))
            emit(lambda _j: (
                V.tensor_tensor(out=gbase[:, cs], in0=gbase[:, cs],
                                in1=EOFF[:, cs], op=AO.add)
            ))
            emit(lambda _j: (
                V.tensor_tensor(out=gdv[:, cs], in0=givB[:, cs], in1=givA[:, cs],
                                op=AO.subtract)
            ))
            emit(lambda _j: (
                V.tensor_scalar(out=gdiff[:, cs], in0=gdv[:, cs], scalar1=20000.0,
                                scalar2=32.0, op0=AO.mult, op1=AO.add)
            ))
            emit(lambda _j: (
                V.scalar_tensor_tensor(out=gidxA[:, cs], in0=givA[:, cs], scalar=20000.0,
                                       in1=gbase[:, cs], op0=AO.mult, op1=AO.add)
            ))
            for j in range(7):
                emit(lambda j: (
                    V.tensor_scalar(out=giw[:, cs], in0=gm0[:, cs],
                                    scalar1=float(32 - j), scalar2=None, op0=AO.is_ge)
                ), j)
                emit(lambda _j: (
                    V.tensor_tensor(out=gt[:, cs], in0=giw[:, cs], in1=gdiff[:, cs],
                                    op=AO.mult)
                ))
                emit(lambda j: (
                    V.scalar_tensor_tensor(out=idxR[b][:, cs, j], in0=gt[:, cs],
                                           scalar=float(-j), in1=gidxA[:, cs],
                                           op0=AO.add, op1=AO.add)
                ), j)

        # -------------- interleaved ring + chain ----------------
        NRC = NCH // 4 + 1
        ringI = pp.tile([128, 2 * NRC], BF16)
        V.memset(ringI[:], 0.0)

        blkR_u16 = [blkR[b][:].bitcast(U16) for b in range(B_LOC)]

        traws = {}
        t4s_map = {}

        def scatter_group(g):
            pair = []
            for b in range(B_LOC):
                traw = traw_pool.tile([128, 32 * (GS + 2)], BF16, name="traw",
                                      tag=f"traw{b}")
                G.local_scatter(
                    out_ap=traw[:].bitcast(U16),
                    data_ap=blkR_u16[b][:, GS * g:GS * (g + 1), :]
                    .rearrange("p c j -> p (c j)"),
                    idxs_ap=idxR[b][:, GS * g:GS * (g + 1), :]
                    .rearrange("p c j -> p (c j)"),
                    channels=128, num_elems=32 * (GS + 2), num_idxs=7 * GS,
                )
                pair.append(traw)
            traws[g] = pair
            t4s_map[g] = [t4_pool.tile([128, 32 * (GS + 2)], BF16, name="t4",
                                       tag=f"t4{b}") for b in range(B_LOC)]

        HW = 32 * (GS + 2) // 2  # transpose half-width (multiple of 32)

        def transpose_half(g, b, h):
            cs = slice(h * HW, (h + 1) * HW)
            nc.vector.transpose(out=t4s_map[g][b][:, cs], in_=traws[g][b][:, cs])

        for b in range(B_LOC):
            idx_math(b, 0)
        pending = []
        for sl in range(1, NSL):
            for b in range(B_LOC):
                idx_math(b, sl, sink=pending)
        pending.reverse()

        for g0 in range(2):
            scatter_group(g0)
            for b in range(B_LOC):
                for h in range(2):
                    transpose_half(g0, b, h)

        # incremental output transpose: segment k (ynat rows 32k..32k+32,
        # i.e. ring groups 32k..32k+32) is transposed+copied as soon as its
        # ring columns are final, so only the last segment remains at the end
        ynat = [pp.tile([NP, 128], F32, name=f"ynat{b}", tag=f"ynat{b}")
                for b in range(B_LOC)]

        def emit_out_segment(k, b, g0):
            src = (ringI[32 * g0:32 * g0 + 32, 2 + 64 * k:2 + 64 * k + 64]
                   .rearrange("p (P s) -> p P s", s=2)[:, :, b])
            ps = psum_t.tile([128, 32], BF16, name="ps_o", tag="ps_t")
            nc.tensor.transpose(ps[32 * k:32 * k + 32, :], src,
                                ident4b[32 * g0:32 * g0 + 32, :],
                                tile_position=(32 * g0, 32 * k))
            dst = ynat[b][32 * k:32 * k + 32, 32 * g0:32 * g0 + 32]
            if b == 0:
                V.tensor_copy(out=dst, in_=ps[32 * k:32 * k + 32, :])
            else:
                SC.activation(out=dst, in_=ps[32 * k:32 * k + 32, :],
                              func=AF.Copy, bias=0.0, scale=1.0)

        def emit_out_dma(k, b):
            nc.sync.dma_start(
                out=out_d[b].rearrange("(P j) -> P j", j=128)[32 * k:32 * k + 32, :],
                in_=ynat[b][32 * k:32 * k + 32, :],
            )

        for g in range(NG):
            t4s = t4s_map.pop(g)
            for s in range(GS):
                c = GS * g + s
                gg, t = c // 4, c % 4
                for b in range(B_LOC):
                    ps = psum_c.tile([128, 1], F32, name=f"ps{b}", tag=f"ps{b}")
                    pieces = PIECES[t]
                    for i, (rb, rs, cd, cb) in enumerate(pieces):
                        rows = slice(rb, rb + rs)
                        nu = gg + cd
                        col0 = (32 * GS + 32 * ((s - 1) // 4)
                                if cb == "X" else 32 * s)
                        nc.tensor.matmul(
                            ps[32 * t:32 * t + 32, 0:1],
                            t4s[b][rows, col0:col0 + 32],
                            ringI[rows, 2 * nu + b:2 * nu + b + 1],
                            start=(i == 0), stop=(i == len(pieces) - 1),
                            tile_position=(rb, 32 * t),
                        )
                    ring_dst = ringI[32 * t:32 * t + 32,
                                     2 * (1 + gg) + b:2 * (1 + gg) + b + 1]
                    x_src = xI[32 * t:32 * t + 32, 2 * gg + b:2 * gg + b + 1]
                    # alternate evac engine per chunk so each chain averages
                    # the fast-DVE and slow-ACT round-trip latencies
                    if (b + c) % 2 == 0:
                        V.scalar_tensor_tensor(
                            out=ring_dst, in0=ps[32 * t:32 * t + 32, 0:1],
                            scalar=1.0, in1=x_src, op0=AO.mult, op1=AO.add,
                        )
                    else:
                        SC.add(out=ring_dst, in_=ps[32 * t:32 * t + 32, 0:1],
                               add=x_src)
                for _ in range(2):
                    if pending:
                        pending.pop()()
                gn = g + 2
                if gn < NG:
                    if s == 1:
                        scatter_group(gn)
                    elif s in (3, 4, 5, 6):
                        bb, hh = divmod(s - 3, 2)
                        transpose_half(gn, bb, hh)
                if g % 16 == 0 and g >= 16:
                    emit_out_segment(g // 16 - 1, s % 2, s // 2)
                elif g % 16 == 1 and g >= 16 and s < 2:
                    emit_out_dma(g // 16 - 1, s)

        # ---------------- final output segment + store ----------------
        for b in range(B_LOC):
            for g0 in range(4):
                emit_out_segment(3, b, g0)
        for b in range(B_LOC):
            emit_out_dma(3, b)


def build_program(N=N_FULL):
    nc = bacc.Bacc("TRN2", target_bir_lowering=False, debug=False,
                   enable_asserts=False)
    f0_d = nc.dram_tensor("f0", [B_LOC, N], F32, kind="ExternalInput").ap()
    x_d = nc.dram_tensor("x", [B_LOC, N], F32, kind="ExternalInput").ap()
    lb_d = nc.dram_tensor("l_b", [B_LOC, N, 2], F32, kind="ExternalInput").ap()
    out_d = nc.dram_tensor("out", [B_LOC, N], F32, kind="ExternalOutput").ap()
    with tile.TileContext(nc) as tc:
        build_kernel(tc, out_d, f0_d, x_d, lb_d, N)
    nc.compile()
    return nc


_PROGRAM_CACHE = {}


def _get_program(N=N_FULL):
    if N not in _PROGRAM_CACHE:
        _PROGRAM_CACHE[N] = build_program(N)
    return _PROGRAM_CACHE[N]


def kernel(f0, x, l_b, K=108, **kwargs):
    """Full-input entry point: shards batch across 8 cores, returns full output."""
    f0 = np.asarray(f0, dtype=np.float32)
    x = np.asarray(x, dtype=np.float32)
    l_b = np.asarray(l_b, dtype=np.float32)
    B, N = x.shape
    assert B == B_FULL and int(K) == 108
    nc = _get_program(N)
    in_maps = []
    for i in range(NCORES):
        sl = slice(i * B_LOC, (i + 1) * B_LOC)
        in_maps.append({
            "f0": np.ascontiguousarray(f0[sl]),
            "x": np.ascontiguousarray(x[sl]),
            "l_b": np.ascontiguousarray(l_b[sl]),
        })
    res = bass_utils.run_bass_kernel_spmd(nc, in_maps, core_ids=list(range(NCORES)))
    out = np.concatenate([res.results[i]["out"] for i in range(NCORES)], axis=0)
    return out.astype(np.float32)



# revision 37
# speedup vs baseline: 1.0751x; 1.0011x over previous
"""Trainium2 Bass kernel for DiffKS (differentiable Karplus-Strong string).

Math (per sequence b, time n):
    g = 0.99*l_b[...,0]; p = l_b[...,1]
    b0 = g*(1-p); a1 = g*p
    f0c = f0 - a1/(b0+a1+1e-7)
    z = floor(f0c); zc = z-2; alpha = f0c - zc
    w_j = Lagrange weights (order 5), j=0..5
    block_j = b0*w_j + a1*w_{j-1}, j=0..6           (7 taps)
    taps live at k = c0+j, c0 = zc-1 = z-3 in [36, 96]
    y[n] = x[n] + sum_j block_j[n] * y[n-1-(c0[n]+j)]    (delays 37..103)

Key structure: minimum delay is 37 > 32, so 32-sample chunks are internally
parallel.  Chunk c is computed as accumulating PE matmuls against the previous
4 chunks' outputs, with per-chunk tap matrices built on-chip by a GPSIMD
local_scatter + DVE 32x32 block transpose.  B=16 is sharded 2 seqs/core.

Phase-1 optimized layout vs the original baseline:
  - both sequences share one interleaved ring tile ringI[128, 2*(NCH/4+1)]
    (col 2*nu+b) and one psum tile [128, 2] per chunk, so each chunk needs a
    single [32,2] DVE evac instead of two.
  - matmul pieces with contiguous rows are merged (avg 1.75 vs 2.25 per
    chunk per seq).
  - natural->S-plane transposes are done as 16 full 128x128 PE transposes
    plus 4-replication matmuls with shared stationary (Rep_rho), evacuated
    with strided copies split between DVE and ACT.
  - scatter index math is reduced (~42 ops/seq) and runs on GPSIMD,
    overlapped with the tap math / transposes, sliced so scatters start
    before all index math finishes.

Layouts (per core, seqs b=0,1; chunk T=32; NCH = N/32 chunks; NP = N/128):
  natural plane  nat[P, b*128+j]  = q[b, 128*P + j]          [NP, 256]
  S-plane        qS[32*rho+f, c]  = q[b, 32*c + f], c = 4P+rho (replicated
                 over rho for scatter source planes)          [128, NCH]
  ring           ringI[32*(c%4)+f, 2*(1+c//4)+b] = y[b, 32*c+f]
Tap matrix for chunk c (lhsT for the PE matmul): rows 32*fl + (31 - m)
address the ring window column; scatter writes single u16s of bf16 taps.

Phase-2 (this session): chain data in bf16 (taps + ring; psum accumulation
stays fp32) -> single-pass PE matmuls instead of fp32 LOW/HIGH, half the
scatter indices, 2x faster DVE transposes; chain evacs split DVE (seq 0) /
ACT (seq 1) so the two evacs run concurrently and ACT is off the DVE queue.
Verified offline: bf16 taps+ring gives ~2e-3 rel err (budget 2e-2).
"""

import numpy as np

import concourse.bass as bass
import concourse.mybir as mybir
import concourse.bacc as bacc
import concourse.tile as tile
from concourse import bass_utils

F32 = mybir.dt.float32
BF16 = mybir.dt.bfloat16
I32 = mybir.dt.int32
I16 = mybir.dt.int16
U16 = mybir.dt.uint16
AO = mybir.AluOpType
AF = mybir.ActivationFunctionType

B_FULL = 16
N_FULL = 16384
NCORES = 8
B_LOC = 2  # sequences per core
GS = 8     # chunks per scatter group

# matmul piece tables per t=c%4: (row_base, row_size, col_delta); ring column
# read is (c//4) + col_delta.  Contiguous same-col-delta rows are merged where
# tile_position allows (row base 0 for sizes > 64); the tile's row space is
# shared between col deltas, so pieces must never overlap rows.
# col base None = main region (32*s); "X" = extra region for t=1's c-1 piece
PIECES = {
    0: [(0, 128, 0, None)],
    1: [(0, 128, 0, None), (0, 32, 1, "X")],
    2: [(64, 64, 0, None), (0, 64, 1, None)],
    3: [(96, 32, 0, None), (0, 96, 1, None)],
}

# Lagrange denominators 1/d_j for order 5
INV_D = [-1.0 / 120, 1.0 / 24, -1.0 / 12, 1.0 / 12, -1.0 / 24, 1.0 / 120]


def build_kernel(tc, out_d, f0_d, x_d, lb_d, N):
    nc = tc.nc
    NP = N // 128          # natural-plane columns per seq
    NCH = N // 32          # chunks per seq
    NG = NCH // GS         # scatter groups
    assert NP * 128 == N and NP == 128 and NG * GS == NCH

    import contextlib
    ctx = contextlib.ExitStack()
    pp = ctx.enter_context(tc.tile_pool(name="persist", bufs=1))
    traw_pool = ctx.enter_context(tc.tile_pool(name="traw", bufs=4))
    t4_pool = ctx.enter_context(tc.tile_pool(name="t4", bufs=12))
    psum_t = ctx.enter_context(tc.tile_pool(name="psum_t", bufs=2, space="PSUM"))
    psum_r = psum_t
    psum_c = ctx.enter_context(tc.tile_pool(name="psum_c", bufs=3, space="PSUM"))

    V = nc.vector
    G = nc.gpsimd
    SC = nc.scalar

    with ctx:
        # ---------------- phase 0: load + elementwise tap math ----------------
        nat_f0 = pp.tile([NP, 256], F32)
        nat_x = pp.tile([NP, 256], F32)
        nat_lb = pp.tile([NP, 512], F32)
        for b in range(B_LOC):
            nc.sync.dma_start(
                out=nat_f0[:, b * 128:(b + 1) * 128],
                in_=f0_d[b].rearrange("(p j) -> p j", j=128),
            )
            nc.sync.dma_start(
                out=nat_x[:, b * 128:(b + 1) * 128],
                in_=x_d[b].rearrange("(p j) -> p j", j=128),
            )
            nc.sync.dma_start(
                out=nat_lb[:, b * 256:(b + 1) * 256],
                in_=lb_d[b].rearrange("(p j) s -> p (j s)", j=128),
            )
        lb_r = nat_lb[:].rearrange("p (j s) -> p j s", s=2)
        g_ap = lb_r[:, :, 0]
        p_ap = lb_r[:, :, 1]

        g99 = pp.tile([NP, 256], F32)
        t_gp = pp.tile([NP, 256], F32)   # a1 = 0.99*g*p
        b0t = pp.tile([NP, 256], F32)
        rec = pp.tile([NP, 256], F32)
        f0c = pp.tile([NP, 256], F32)
        zf = pp.tile([NP, 256], F32)
        tmp1 = pp.tile([NP, 256], F32)
        tmp2 = pp.tile([NP, 256], F32)
        itmp = pp.tile([NP, 256], I32)

        # zf chain first so its transposes (and gpsimd index math) start early
        V.tensor_scalar(out=g99[:], in0=g_ap, scalar1=0.99, scalar2=None, op0=AO.mult)
        V.tensor_tensor(out=t_gp[:], in0=g99[:], in1=p_ap, op=AO.mult)       # a1
        V.tensor_tensor(out=b0t[:], in0=g99[:], in1=t_gp[:], op=AO.subtract)  # b0
        V.tensor_scalar(out=tmp1[:], in0=g99[:], scalar1=1e-7, scalar2=None, op0=AO.add)
        V.reciprocal(out=rec[:], in_=tmp1[:])
        V.tensor_tensor(out=tmp2[:], in0=t_gp[:], in1=rec[:], op=AO.mult)
        V.tensor_tensor(out=f0c[:], in0=nat_f0[:], in1=tmp2[:], op=AO.subtract)
        # zf = floor(f0c), robust to cast rounding mode
        V.tensor_copy(out=itmp[:], in_=f0c[:])
        V.tensor_copy(out=zf[:], in_=itmp[:])
        V.tensor_tensor(out=tmp1[:], in0=zf[:], in1=f0c[:], op=AO.is_gt)
        V.tensor_tensor(out=zf[:], in0=zf[:], in1=tmp1[:], op=AO.subtract)

        # ---- transpose machinery (identity + replication stationaries) ----
        ident = pp.tile([128, 128], F32)
        G.memset(ident[:], 1.0)
        G.affine_select(out=ident[:], in_=ident[:], pattern=[[1, 128]],
                        compare_op=AO.is_equal, fill=0.0, base=0,
                        channel_multiplier=-1)
        identb = pp.tile([128, 128], BF16)
        G.memset(identb[:], 1.0)
        G.affine_select(out=identb[:], in_=identb[:], pattern=[[1, 128]],
                        compare_op=AO.is_equal, fill=0.0, base=0,
                        channel_multiplier=-1)
        ident4b = pp.tile([128, 32], BF16)
        G.memset(ident4b[:], 1.0)
        for g0 in range(4):
            G.affine_select(out=ident4b[32 * g0:32 * g0 + 32, :],
                            in_=ident4b[32 * g0:32 * g0 + 32, :],
                            pattern=[[1, 32]], compare_op=AO.is_equal,
                            fill=0.0, base=0, channel_multiplier=-1)
        # Rep[rho]: [128,128] with identity blocks in rows 32rho..32rho+32 at
        # every 32-col block: out = Rep[rho].T @ T replicates quadrant rho.
        Rep = []
        Repb = []
        for rho in range(4):
            for lst, dt, nm in ((Rep, F32, "rep"), (Repb, BF16, "repb")):
                R = pp.tile([128, 128], dt, name=f"{nm}{rho}", tag=f"{nm}{rho}")
                G.memset(R[:], 0.0)
                G.memset(R[32 * rho:32 * rho + 32, :], 1.0)
                for q in range(4):
                    G.affine_select(
                        out=R[32 * rho:32 * rho + 32, 32 * q:32 * q + 32],
                        in_=R[32 * rho:32 * rho + 32, 32 * q:32 * q + 32],
                        pattern=[[1, 32]], compare_op=AO.is_equal,
                        fill=0.0, base=0, channel_multiplier=-1)
                lst.append(R)

        # ---- gpsimd one-time index tiles ----
        fi = pp.tile([128, 1], I32)
        G.iota(fi[:], pattern=[[1, 1]], base=0, channel_multiplier=1)
        ff = pp.tile([128, 1], F32)
        V.tensor_copy(out=ff[:], in_=fi[:])
        s1 = pp.tile([128, 1], F32)
        s2 = pp.tile([128, 1], F32)
        i1 = pp.tile([128, 1], I32)
        V.tensor_scalar(out=s1[:], in0=ff[:], scalar1=1.0 / 32, scalar2=None, op0=AO.mult)
        V.tensor_copy(out=i1[:], in_=s1[:])
        V.tensor_copy(out=s2[:], in_=i1[:])          # rho = p//32
        s3 = pp.tile([128, 1], F32)
        V.tensor_tensor(out=s3[:], in0=s2[:], in1=s1[:], op=AO.is_gt)
        V.tensor_tensor(out=s2[:], in0=s2[:], in1=s3[:], op=AO.subtract)
        fmod = pp.tile([128, 1], F32)
        V.scalar_tensor_tensor(out=fmod[:], in0=s2[:], scalar=-32.0, in1=ff[:],
                               op0=AO.mult, op1=AO.add)   # f = p%32
        sc0 = pp.tile([128, 1], F32)
        V.tensor_scalar(out=sc0[:], in0=fmod[:], scalar1=-1.0, scalar2=-3.0,
                        op0=AO.mult, op1=AO.add)          # -3 - f
        rho_f = s2
        rho1_f = pp.tile([128, 1], F32)                   # (rho+1)%4
        V.tensor_scalar(out=rho1_f[:], in0=rho_f[:], scalar1=1.0, scalar2=None, op0=AO.add)
        w4t = pp.tile([128, 1], F32)
        V.tensor_scalar(out=w4t[:], in0=rho1_f[:], scalar1=4.0, scalar2=None, op0=AO.is_ge)
        V.scalar_tensor_tensor(out=rho1_f[:], in0=w4t[:], scalar=-4.0, in1=rho1_f[:],
                               op0=AO.mult, op1=AO.add)

        itc = pp.tile([128, NCH], I32)
        G.iota(itc[:], pattern=[[0, NCH // 4], [1, 4]], base=3,
               channel_multiplier=0)
        tcol3 = pp.tile([128, NCH], F32)                  # c%4 + 3
        V.tensor_copy(out=tcol3[:], in_=itc[:])
        itc2 = pp.tile([128, NCH], I32)
        G.iota(itc2[:], pattern=[[0, NCH // GS], [32, GS]], base=31 - 20000,
               channel_multiplier=0)
        S_c = pp.tile([128, NCH], F32)                    # 32*(c%GS) + 31 - 20000
        V.tensor_copy(out=S_c[:], in_=itc2[:])
        # EOFF: moves t=1 chunks' source-(c-1) taps (all valid taps at rho==0,
        # c%4==1) into the per-group extra scatter region: u16 offset from the
        # main base 32*(c%GS): s=1 -> 256-32 = +224, s=5 -> 256+32-160 = +128.
        itc3 = pp.tile([128, NCH], I32)
        G.iota(itc3[:], pattern=[[0, NCH // GS], [1, GS]], base=0,
               channel_multiplier=0)
        c8f = pp.tile([128, NCH], F32)
        V.tensor_copy(out=c8f[:], in_=itc3[:])
        e1 = pp.tile([128, NCH], F32)
        e5 = pp.tile([128, NCH], F32)
        V.tensor_scalar(out=e1[:], in0=c8f[:], scalar1=1.0, scalar2=224.0,
                        op0=AO.is_equal, op1=AO.mult)
        V.tensor_scalar(out=e5[:], in0=c8f[:], scalar1=5.0, scalar2=128.0,
                        op0=AO.is_equal, op1=AO.mult)
        EOFF = pp.tile([128, NCH], F32)
        V.memset(EOFF[:], 0.0)
        V.tensor_tensor(out=EOFF[0:32, :], in0=e1[0:32, :], in1=e5[0:32, :],
                        op=AO.add)

        # ------- natural [NP,128] -> replicated S-plane [128,NCH] -------
        # pairs: 0..1 = zf (fp32); T_blk holds the 14 blk pairs in bf16
        T_all = pp.tile([128, 2 * 128], F32)
        T_blk = pp.tile([128, 14 * 128], BF16)

        zfR = [pp.tile([128, NCH], F32, name=f"zfR{b}", tag=f"zfR{b}")
               for b in range(B_LOC)]
        blkR = [pp.tile([128, NCH, 7], BF16, name=f"blkR{b}", tag=f"blkR{b}")
                for b in range(B_LOC)]
        xI = pp.tile([128, 2 * NP], F32)   # transposed x, col 2*P+... seq-interleaved

        def nat_transpose(src_ap, k, engine, bf=False):
            """[NP,128] natural block -> (T_blk if bf else T_all) pair k."""
            dt = BF16 if bf else F32
            ps = psum_t.tile([128, 128], dt, name="ps_t", tag="ps_t")
            nc.tensor.transpose(ps[:], src_ap, (identb if bf else ident)[:])
            dst_t = T_blk if bf else T_all
            dst = dst_t[:, 128 * k:128 * (k + 1)]
            if engine == 0:
                V.tensor_copy(out=dst, in_=ps[:])
            else:
                SC.activation(out=dst, in_=ps[:],
                              func=AF.Copy, bias=0.0, scale=1.0)

        def rep_evac(kk, rho, psR, off, engine):
            """psR[:, 128*off:...] -> strided S-plane destination for pair kk.

            Split by P-half: ACT writes P<64 (chunks<256, needed first by the
            chain) and DVE writes P>=64 (needed only after chunk 256), so the
            two evac chains run concurrently and the scatter unblocks early.
            """
            b, q = kk % 2, kk // 2
            if q == 0:
                dst = zfR[b][:].rearrange("p (P r) -> p P r", r=4)[:, :, rho]
            else:
                dst = (blkR[b][:].rearrange("p (P r) j -> p P r j", r=4)
                       [:, :, rho, q - 1])
            src = psR[:, 128 * off:128 * (off + 1)]
            SC.activation(out=dst[:, 0:64], in_=src[:, 0:64],
                          func=AF.Copy, bias=0.0, scale=1.0)
            V.tensor_copy(out=dst[:, 64:128], in_=src[:, 64:128])

        # zf transposes first (pairs k=0,1), then replicate -> zfR early
        for b in range(B_LOC):
            nat_transpose(zf[:, b * 128:(b + 1) * 128], b, engine=1)
        for rho in range(4):
            psR = psum_r.tile([128, 256], F32, name="ps_rz", tag="ps_t")
            nc.tensor.matmul(psR[:], Rep[rho][:], T_all[:, 0:256],
                             start=True, stop=True)
            for kk in range(2):
                rep_evac(kk, rho, psR, kk, engine=1)

        # remaining tap math on DVE while zf replication + index math proceed
        D = f0c
        V.tensor_tensor(out=D[:], in0=f0c[:], in1=zf[:], op=AO.subtract)
        u = [pp.tile([NP, 256], F32, name=f"u{m}", tag=f"u{m}") for m in range(6)]
        for m in range(6):
            V.tensor_scalar(out=u[m][:], in0=D[:], scalar1=float(2 - m),
                            scalar2=None, op0=AO.add)
        pre = [None] * 6
        suf = [None] * 7
        pre[1] = u[0]
        for j in range(2, 6):
            pre[j] = pp.tile([NP, 256], F32, name=f"pre{j}", tag=f"pre{j}")
            V.tensor_tensor(out=pre[j][:], in0=pre[j - 1][:], in1=u[j - 1][:], op=AO.mult)
        suf[5] = u[5]
        for j in range(4, 0, -1):
            suf[j] = pp.tile([NP, 256], F32, name=f"suf{j}", tag=f"suf{j}")
            V.tensor_tensor(out=suf[j][:], in0=suf[j + 1][:], in1=u[j][:], op=AO.mult)
        w = [pp.tile([NP, 256], F32, name=f"w{j}", tag=f"w{j}") for j in range(6)]
        V.tensor_scalar(out=w[0][:], in0=suf[1][:], scalar1=INV_D[0], scalar2=None, op0=AO.mult)
        for j in range(1, 5):
            V.scalar_tensor_tensor(out=w[j][:], in0=pre[j][:], scalar=INV_D[j],
                                   in1=suf[j + 1][:], op0=AO.mult, op1=AO.mult)
        V.tensor_scalar(out=w[5][:], in0=pre[5][:], scalar1=INV_D[5], scalar2=None, op0=AO.mult)

        blk = [pp.tile([NP, 256], BF16, name=f"blk{j}", tag=f"blk{j}") for j in range(7)]
        V.tensor_tensor(out=blk[0][:], in0=b0t[:], in1=w[0][:], op=AO.mult)
        for j in range(1, 6):
            V.tensor_tensor(out=tmp2[:], in0=b0t[:], in1=w[j][:], op=AO.mult)
            V.tensor_tensor(out=tmp1[:], in0=t_gp[:], in1=w[j - 1][:], op=AO.mult)
            V.tensor_tensor(out=blk[j][:], in0=tmp2[:], in1=tmp1[:], op=AO.add)
        V.tensor_tensor(out=blk[6][:], in0=t_gp[:], in1=w[5][:], op=AO.mult)

        # blk + x transposes
        for j in range(7):
            for b in range(B_LOC):
                nat_transpose(blk[j][:, b * 128:(b + 1) * 128], 2 * j + b,
                              engine=1, bf=True)
        for b in range(B_LOC):
            ps = psum_t.tile([128, 128], F32, name="ps_x", tag="ps_t")
            nc.tensor.transpose(ps[:], nat_x[:, b * 128:(b + 1) * 128], ident[:])
            V.tensor_copy(out=xI[:].rearrange("p (P s) -> p P s", s=2)[:, :, b],
                          in_=ps[:])
        # replicate blk planes (bf16 path: 1 cycle/row on the PE)
        for rho in range(4):
            for col0, npair in ((0, 4), (512, 4), (1024, 4), (1536, 2)):
                psR = psum_r.tile([128, 128 * npair], F32, name="ps_rb", tag="ps_t")
                nc.tensor.matmul(psR[:], Repb[rho][:],
                                 T_blk[:, col0:col0 + 128 * npair],
                                 start=True, stop=True)
                for kk in range(npair):
                    rep_evac(2 + col0 // 128 + kk, rho, psR, kk,
                             engine=1)

        # ---------------- scatter index computation (GPSIMD) ----------------
        # v0 = zf - 3 - f; fl = v0//32; m = v0%32; uB = c%4 + 3 - fl
        # no-wrap valid: (uB%4 == rho); wrap valid: (uB%4 == (rho+1)%4)
        # idx[j] = valid_j*20000 + 32*iw_j + 31 - m - j - 20000 + 32*(c%GS)
        idxR = [pp.tile([128, NCH, 7], I16, name=f"idxR{b}", tag=f"idxR{b}")
                for b in range(B_LOC)]
        gv0 = pp.tile([128, NCH], F32)
        gtA = pp.tile([128, NCH], F32)
        gfl_i = pp.tile([128, NCH], I16)
        gflf = pp.tile([128, NCH], F32)
        gm0 = pp.tile([128, NCH], F32)
        guB = pp.tile([128, NCH], F32)
        gw4 = pp.tile([128, NCH], F32)
        givA = pp.tile([128, NCH], F32)
        givB = pp.tile([128, NCH], F32)
        gbase = pp.tile([128, NCH], F32)
        gdv = pp.tile([128, NCH], F32)
        gdiff = pp.tile([128, NCH], F32)
        gidxA = pp.tile([128, NCH], F32)
        giw = pp.tile([128, NCH], F32)
        gt = pp.tile([128, NCH], F32)
        gtj = pp.tile([128, NCH], F32)

        NSL = 4                       # column slices for early scatter start
        SW = NCH // NSL

        def idx_math(b, sl, sink=None):
            def emit(f, j=None):
                if sink is None:
                    f(j)
                else:
                    sink.append(lambda jj=j: f(jj))
            cs = slice(sl * SW, (sl + 1) * SW)
            emit(lambda _j: (
                V.tensor_scalar(out=gv0[:, cs], in0=zfR[b][:, cs], scalar1=sc0[:],
                                scalar2=None, op0=AO.add)
            ))
            emit(lambda _j: (
                V.tensor_scalar(out=gtA[:, cs], in0=gv0[:, cs], scalar1=1.0 / 32,
                                scalar2=None, op0=AO.mult)
            ))
            emit(lambda _j: (
                V.tensor_copy(out=gfl_i[:, cs], in_=gtA[:, cs])
            ))
            emit(lambda _j: (
                V.tensor_copy(out=gflf[:, cs], in_=gfl_i[:, cs])
            ))
            emit(lambda _j: (
                V.tensor_tensor(out=gw4[:, cs], in0=gflf[:, cs], in1=gtA[:, cs],
                                op=AO.is_gt)
            ))
            emit(lambda _j: (
                V.tensor_tensor(out=gflf[:, cs], in0=gflf[:, cs], in1=gw4[:, cs],
                                op=AO.subtract)
            ))
            emit(lambda _j: (
                V.scalar_tensor_tensor(out=gm0[:, cs], in0=gflf[:, cs], scalar=-32.0,
                                       in1=gv0[:, cs], op0=AO.mult, op1=AO.add)
            ))
            emit(lambda _j: (
                V.scalar_tensor_tensor(out=guB[:, cs], in0=gflf[:, cs], scalar=-1.0,
                                       in1=tcol3[:, cs], op0=AO.mult, op1=AO.add)
            ))
            emit(lambda _j: (
                V.tensor_scalar(out=gw4[:, cs], in0=guB[:, cs], scalar1=4.0,
                                scalar2=None, op0=AO.is_ge)
            ))
            emit(lambda _j: (
                V.scalar_tensor_tensor(out=guB[:, cs], in0=gw4[:, cs], scalar=-4.0,
                                       in1=guB[:, cs], op0=AO.mult, op1=AO.add)
            ))
            emit(lambda _j: (
                V.tensor_scalar(out=givA[:, cs], in0=guB[:, cs], scalar1=rho_f[:],
                                scalar2=None, op0=AO.is_equal)
            ))
            emit(lambda _j: (
                V.tensor_scalar(out=givB[:, cs], in0=guB[:, cs], scalar1=rho1_f[:],
                                scalar2=None, op0=AO.is_equal)
            ))
            emit(lambda _j: (
                V.scalar_tensor_tensor(out=gbase[:, cs], in0=gm0[:, cs], scalar=-1.0,
                                       in1=S_c[:, cs], op0=AO.mult, op1=AO.add)
            ))
            emit(lambda _j: (
                V.tensor_tensor(out=gbase[:, cs], in0=gbase[:, cs],
                                in1=EOFF[:, cs], op=AO.add)
            ))
            emit(lambda _j: (
                V.tensor_tensor(out=gdv[:, cs], in0=givB[:, cs], in1=givA[:, cs],
                                op=AO.subtract)
            ))
            emit(lambda _j: (
                V.tensor_scalar(out=gdiff[:, cs], in0=gdv[:, cs], scalar1=20000.0,
                                scalar2=32.0, op0=AO.mult, op1=AO.add)
            ))
            emit(lambda _j: (
                V.scalar_tensor_tensor(out=gidxA[:, cs], in0=givA[:, cs], scalar=20000.0,
                                       in1=gbase[:, cs], op0=AO.mult, op1=AO.add)
            ))
            for j in range(7):
                emit(lambda j: (
                    V.tensor_scalar(out=giw[:, cs], in0=gm0[:, cs],
                                    scalar1=float(32 - j), scalar2=None, op0=AO.is_ge)
                ), j)
                emit(lambda _j: (
                    V.tensor_tensor(out=gt[:, cs], in0=giw[:, cs], in1=gdiff[:, cs],
                                    op=AO.mult)
                ))
                emit(lambda j: (
                    V.scalar_tensor_tensor(out=idxR[b][:, cs, j], in0=gt[:, cs],
                                           scalar=float(-j), in1=gidxA[:, cs],
                                           op0=AO.add, op1=AO.add)
                ), j)

        # -------------- interleaved ring + chain ----------------
        NRC = NCH // 4 + 1
        ringI = pp.tile([128, 2 * NRC], BF16)
        V.memset(ringI[:], 0.0)

        blkR_u16 = [blkR[b][:].bitcast(U16) for b in range(B_LOC)]

        traws = {}
        t4s_map = {}

        def scatter_group(g):
            pair = []
            for b in range(B_LOC):
                traw = traw_pool.tile([128, 32 * (GS + 2)], BF16, name="traw",
                                      tag=f"traw{b}")
                G.local_scatter(
                    out_ap=traw[:].bitcast(U16),
                    data_ap=blkR_u16[b][:, GS * g:GS * (g + 1), :]
                    .rearrange("p c j -> p (c j)"),
                    idxs_ap=idxR[b][:, GS * g:GS * (g + 1), :]
                    .rearrange("p c j -> p (c j)"),
                    channels=128, num_elems=32 * (GS + 2), num_idxs=7 * GS,
                )
                pair.append(traw)
            traws[g] = pair
            t4s_map[g] = [t4_pool.tile([128, 32 * (GS + 2)], BF16, name="t4",
                                       tag=f"t4{b}") for b in range(B_LOC)]

        HW = 32 * (GS + 2) // 2  # transpose half-width (multiple of 32)

        def transpose_half(g, b, h):
            cs = slice(h * HW, (h + 1) * HW)
            nc.vector.transpose(out=t4s_map[g][b][:, cs], in_=traws[g][b][:, cs])

        for b in range(B_LOC):
            idx_math(b, 0)
        pending = []
        for sl in range(1, NSL):
            for b in range(B_LOC):
                idx_math(b, sl, sink=pending)
        pending.reverse()

        for g0 in range(2):
            scatter_group(g0)
            for b in range(B_LOC):
                for h in range(2):
                    transpose_half(g0, b, h)

        # incremental output transpose: segment k (ynat rows 32k..32k+32,
        # i.e. ring groups 32k..32k+32) is transposed+copied as soon as its
        # ring columns are final, so only the last segment remains at the end
        ynat = [pp.tile([NP, 128], F32, name=f"ynat{b}", tag=f"ynat{b}")
                for b in range(B_LOC)]

        def emit_out_segment(k, b, g0):
            src = (ringI[32 * g0:32 * g0 + 32, 2 + 64 * k:2 + 64 * k + 64]
                   .rearrange("p (P s) -> p P s", s=2)[:, :, b])
            ps = psum_t.tile([128, 32], BF16, name="ps_o", tag="ps_t")
            nc.tensor.transpose(ps[32 * k:32 * k + 32, :], src,
                                ident4b[32 * g0:32 * g0 + 32, :],
                                tile_position=(32 * g0, 32 * k))
            dst = ynat[b][32 * k:32 * k + 32, 32 * g0:32 * g0 + 32]
            if b == 0:
                V.tensor_copy(out=dst, in_=ps[32 * k:32 * k + 32, :])
            else:
                SC.activation(out=dst, in_=ps[32 * k:32 * k + 32, :],
                              func=AF.Copy, bias=0.0, scale=1.0)

        def emit_out_dma(k, b):
            nc.sync.dma_start(
                out=out_d[b].rearrange("(P j) -> P j", j=128)[32 * k:32 * k + 32, :],
                in_=ynat[b][32 * k:32 * k + 32, :],
            )

        for g in range(NG):
            t4s = t4s_map.pop(g)
            for s in range(GS):
                c = GS * g + s
                gg, t = c // 4, c % 4
                for b in range(B_LOC):
                    ps = psum_c.tile([128, 1], F32, name=f"ps{b}", tag=f"ps{b}")
                    pieces = PIECES[t]
                    for i, (rb, rs, cd, cb) in enumerate(pieces):
                        rows = slice(rb, rb + rs)
                        nu = gg + cd
                        col0 = (32 * GS + 32 * ((s - 1) // 4)
                                if cb == "X" else 32 * s)
                        nc.tensor.matmul(
                            ps[32 * t:32 * t + 32, 0:1],
                            t4s[b][rows, col0:col0 + 32],
                            ringI[rows, 2 * nu + b:2 * nu + b + 1],
                            start=(i == 0), stop=(i == len(pieces) - 1),
                            tile_position=(rb, 32 * t),
                        )
                    ring_dst = ringI[32 * t:32 * t + 32,
                                     2 * (1 + gg) + b:2 * (1 + gg) + b + 1]
                    x_src = xI[32 * t:32 * t + 32, 2 * gg + b:2 * gg + b + 1]
                    # alternate evac engine per chunk so each chain averages
                    # the fast-DVE and slow-ACT round-trip latencies
                    if (b + c) % 2 == 0:
                        V.scalar_tensor_tensor(
                            out=ring_dst, in0=ps[32 * t:32 * t + 32, 0:1],
                            scalar=1.0, in1=x_src, op0=AO.mult, op1=AO.add,
                        )
                    else:
                        SC.add(out=ring_dst, in_=ps[32 * t:32 * t + 32, 0:1],
                               add=x_src)
                if pending:
                    pending.pop()()
                gn = g + 2
                if gn < NG:
                    if s == 1:
                        scatter_group(gn)
                    elif s in (3, 4, 5, 6):
                        bb, hh = divmod(s - 3, 2)
                        transpose_half(gn, bb, hh)
                if g % 16 == 0 and g >= 16:
                    emit_out_segment(g // 16 - 1, s % 2, s // 2)
                elif g % 16 == 1 and g >= 16 and s < 2:
                    emit_out_dma(g // 16 - 1, s)

        # ---------------- final output segment + store ----------------
        for b in range(B_LOC):
            for g0 in range(4):
                emit_out_segment(3, b, g0)
        for b in range(B_LOC):
            emit_out_dma(3, b)


def build_program(N=N_FULL):
    nc = bacc.Bacc("TRN2", target_bir_lowering=False, debug=False,
                   enable_asserts=False)
    f0_d = nc.dram_tensor("f0", [B_LOC, N], F32, kind="ExternalInput").ap()
    x_d = nc.dram_tensor("x", [B_LOC, N], F32, kind="ExternalInput").ap()
    lb_d = nc.dram_tensor("l_b", [B_LOC, N, 2], F32, kind="ExternalInput").ap()
    out_d = nc.dram_tensor("out", [B_LOC, N], F32, kind="ExternalOutput").ap()
    with tile.TileContext(nc) as tc:
        build_kernel(tc, out_d, f0_d, x_d, lb_d, N)
    nc.compile()
    return nc


_PROGRAM_CACHE = {}


def _get_program(N=N_FULL):
    if N not in _PROGRAM_CACHE:
        _PROGRAM_CACHE[N] = build_program(N)
    return _PROGRAM_CACHE[N]


def kernel(f0, x, l_b, K=108, **kwargs):
    """Full-input entry point: shards batch across 8 cores, returns full output."""
    f0 = np.asarray(f0, dtype=np.float32)
    x = np.asarray(x, dtype=np.float32)
    l_b = np.asarray(l_b, dtype=np.float32)
    B, N = x.shape
    assert B == B_FULL and int(K) == 108
    nc = _get_program(N)
    in_maps = []
    for i in range(NCORES):
        sl = slice(i * B_LOC, (i + 1) * B_LOC)
        in_maps.append({
            "f0": np.ascontiguousarray(f0[sl]),
            "x": np.ascontiguousarray(x[sl]),
            "l_b": np.ascontiguousarray(l_b[sl]),
        })
    res = bass_utils.run_bass_kernel_spmd(nc, in_maps, core_ids=list(range(NCORES)))
    out = np.concatenate([res.results[i]["out"] for i in range(NCORES)], axis=0)
    return out.astype(np.float32)



# revision 40
# speedup vs baseline: 1.0791x; 1.0037x over previous
"""Trainium2 Bass kernel for DiffKS (differentiable Karplus-Strong string).

Math (per sequence b, time n):
    g = 0.99*l_b[...,0]; p = l_b[...,1]
    b0 = g*(1-p); a1 = g*p
    f0c = f0 - a1/(b0+a1+1e-7)
    z = floor(f0c); zc = z-2; alpha = f0c - zc
    w_j = Lagrange weights (order 5), j=0..5
    block_j = b0*w_j + a1*w_{j-1}, j=0..6           (7 taps)
    taps live at k = c0+j, c0 = zc-1 = z-3 in [36, 96]
    y[n] = x[n] + sum_j block_j[n] * y[n-1-(c0[n]+j)]    (delays 37..103)

Key structure: minimum delay is 37 > 32, so 32-sample chunks are internally
parallel.  Chunk c is computed as accumulating PE matmuls against the previous
4 chunks' outputs, with per-chunk tap matrices built on-chip by a GPSIMD
local_scatter + DVE 32x32 block transpose.  B=16 is sharded 2 seqs/core.

Phase-1 optimized layout vs the original baseline:
  - both sequences share one interleaved ring tile ringI[128, 2*(NCH/4+1)]
    (col 2*nu+b) and one psum tile [128, 2] per chunk, so each chunk needs a
    single [32,2] DVE evac instead of two.
  - matmul pieces with contiguous rows are merged (avg 1.75 vs 2.25 per
    chunk per seq).
  - natural->S-plane transposes are done as 16 full 128x128 PE transposes
    plus 4-replication matmuls with shared stationary (Rep_rho), evacuated
    with strided copies split between DVE and ACT.
  - scatter index math is reduced (~42 ops/seq) and runs on GPSIMD,
    overlapped with the tap math / transposes, sliced so scatters start
    before all index math finishes.

Layouts (per core, seqs b=0,1; chunk T=32; NCH = N/32 chunks; NP = N/128):
  natural plane  nat[P, b*128+j]  = q[b, 128*P + j]          [NP, 256]
  S-plane        qS[32*rho+f, c]  = q[b, 32*c + f], c = 4P+rho (replicated
                 over rho for scatter source planes)          [128, NCH]
  ring           ringI[32*(c%4)+f, 2*(1+c//4)+b] = y[b, 32*c+f]
Tap matrix for chunk c (lhsT for the PE matmul): rows 32*fl + (31 - m)
address the ring window column; scatter writes single u16s of bf16 taps.

Phase-2 (this session): chain data in bf16 (taps + ring; psum accumulation
stays fp32) -> single-pass PE matmuls instead of fp32 LOW/HIGH, half the
scatter indices, 2x faster DVE transposes; chain evacs split DVE (seq 0) /
ACT (seq 1) so the two evacs run concurrently and ACT is off the DVE queue.
Verified offline: bf16 taps+ring gives ~2e-3 rel err (budget 2e-2).
"""

import numpy as np

import concourse.bass as bass
import concourse.mybir as mybir
import concourse.bacc as bacc
import concourse.tile as tile
from concourse import bass_utils

F32 = mybir.dt.float32
BF16 = mybir.dt.bfloat16
I32 = mybir.dt.int32
I16 = mybir.dt.int16
U16 = mybir.dt.uint16
AO = mybir.AluOpType
AF = mybir.ActivationFunctionType

B_FULL = 16
N_FULL = 16384
NCORES = 8
B_LOC = 2  # sequences per core
GS = 8     # chunks per scatter group

# matmul piece tables per t=c%4: (row_base, row_size, col_delta); ring column
# read is (c//4) + col_delta.  Contiguous same-col-delta rows are merged where
# tile_position allows (row base 0 for sizes > 64); the tile's row space is
# shared between col deltas, so pieces must never overlap rows.
# col base None = main region (32*s); "X" = extra region for t=1's c-1 piece
PIECES = {
    0: [(0, 128, 0, None)],
    1: [(0, 128, 0, None), (0, 32, 1, "X")],
    2: [(64, 64, 0, None), (0, 64, 1, None)],
    3: [(96, 32, 0, None), (0, 96, 1, None)],
}

# Lagrange denominators 1/d_j for order 5
INV_D = [-1.0 / 120, 1.0 / 24, -1.0 / 12, 1.0 / 12, -1.0 / 24, 1.0 / 120]


def build_kernel(tc, out_d, f0_d, x_d, lb_d, N):
    nc = tc.nc
    NP = N // 128          # natural-plane columns per seq
    NCH = N // 32          # chunks per seq
    NG = NCH // GS         # scatter groups
    assert NP * 128 == N and NP == 128 and NG * GS == NCH

    import contextlib
    ctx = contextlib.ExitStack()
    pp = ctx.enter_context(tc.tile_pool(name="persist", bufs=1))
    traw_pool = ctx.enter_context(tc.tile_pool(name="traw", bufs=4))
    t4_pool = ctx.enter_context(tc.tile_pool(name="t4", bufs=12))
    psum_t = ctx.enter_context(tc.tile_pool(name="psum_t", bufs=2, space="PSUM"))
    psum_r = psum_t
    psum_c = ctx.enter_context(tc.tile_pool(name="psum_c", bufs=3, space="PSUM"))

    V = nc.vector
    G = nc.gpsimd
    SC = nc.scalar

    with ctx:
        # ---------------- phase 0: load + elementwise tap math ----------------
        nat_f0 = pp.tile([NP, 256], F32)
        nat_x = pp.tile([NP, 256], F32)
        nat_lb = pp.tile([NP, 512], F32)
        for b in range(B_LOC):
            nc.sync.dma_start(
                out=nat_f0[:, b * 128:(b + 1) * 128],
                in_=f0_d[b].rearrange("(p j) -> p j", j=128),
            )
            nc.sync.dma_start(
                out=nat_x[:, b * 128:(b + 1) * 128],
                in_=x_d[b].rearrange("(p j) -> p j", j=128),
            )
            nc.sync.dma_start(
                out=nat_lb[:, b * 256:(b + 1) * 256],
                in_=lb_d[b].rearrange("(p j) s -> p (j s)", j=128),
            )
        lb_r = nat_lb[:].rearrange("p (j s) -> p j s", s=2)
        g_ap = lb_r[:, :, 0]
        p_ap = lb_r[:, :, 1]

        g99 = pp.tile([NP, 256], F32)
        t_gp = pp.tile([NP, 256], F32)   # a1 = 0.99*g*p
        b0t = pp.tile([NP, 256], F32)
        rec = pp.tile([NP, 256], F32)
        f0c = pp.tile([NP, 256], F32)
        zf = pp.tile([NP, 256], F32)
        tmp1 = pp.tile([NP, 256], F32)
        tmp2 = pp.tile([NP, 256], F32)
        itmp = pp.tile([NP, 256], I32)

        # zf chain first so its transposes (and gpsimd index math) start early
        V.tensor_scalar(out=g99[:], in0=g_ap, scalar1=0.99, scalar2=None, op0=AO.mult)
        V.tensor_tensor(out=t_gp[:], in0=g99[:], in1=p_ap, op=AO.mult)       # a1
        V.tensor_tensor(out=b0t[:], in0=g99[:], in1=t_gp[:], op=AO.subtract)  # b0
        V.tensor_scalar(out=tmp1[:], in0=g99[:], scalar1=1e-7, scalar2=None, op0=AO.add)
        V.reciprocal(out=rec[:], in_=tmp1[:])
        V.tensor_tensor(out=tmp2[:], in0=t_gp[:], in1=rec[:], op=AO.mult)
        V.tensor_tensor(out=f0c[:], in0=nat_f0[:], in1=tmp2[:], op=AO.subtract)
        # zf = floor(f0c), robust to cast rounding mode
        V.tensor_copy(out=itmp[:], in_=f0c[:])
        V.tensor_copy(out=zf[:], in_=itmp[:])
        V.tensor_tensor(out=tmp1[:], in0=zf[:], in1=f0c[:], op=AO.is_gt)
        V.tensor_tensor(out=zf[:], in0=zf[:], in1=tmp1[:], op=AO.subtract)

        # ---- transpose machinery (identity + replication stationaries) ----
        ident = pp.tile([128, 128], F32)
        G.memset(ident[:], 1.0)
        G.affine_select(out=ident[:], in_=ident[:], pattern=[[1, 128]],
                        compare_op=AO.is_equal, fill=0.0, base=0,
                        channel_multiplier=-1)
        identb = pp.tile([128, 128], BF16)
        G.memset(identb[:], 1.0)
        G.affine_select(out=identb[:], in_=identb[:], pattern=[[1, 128]],
                        compare_op=AO.is_equal, fill=0.0, base=0,
                        channel_multiplier=-1)
        ident4b = pp.tile([128, 32], BF16)
        G.memset(ident4b[:], 1.0)
        for g0 in range(4):
            G.affine_select(out=ident4b[32 * g0:32 * g0 + 32, :],
                            in_=ident4b[32 * g0:32 * g0 + 32, :],
                            pattern=[[1, 32]], compare_op=AO.is_equal,
                            fill=0.0, base=0, channel_multiplier=-1)
        # Rep[rho]: [128,128] with identity blocks in rows 32rho..32rho+32 at
        # every 32-col block: out = Rep[rho].T @ T replicates quadrant rho.
        Rep = []
        Repb = []
        for rho in range(4):
            for lst, dt, nm in ((Rep, F32, "rep"), (Repb, BF16, "repb")):
                R = pp.tile([128, 128], dt, name=f"{nm}{rho}", tag=f"{nm}{rho}")
                G.memset(R[:], 0.0)
                G.memset(R[32 * rho:32 * rho + 32, :], 1.0)
                for q in range(4):
                    G.affine_select(
                        out=R[32 * rho:32 * rho + 32, 32 * q:32 * q + 32],
                        in_=R[32 * rho:32 * rho + 32, 32 * q:32 * q + 32],
                        pattern=[[1, 32]], compare_op=AO.is_equal,
                        fill=0.0, base=0, channel_multiplier=-1)
                lst.append(R)

        # ---- gpsimd one-time index tiles ----
        fi = pp.tile([128, 1], I32)
        G.iota(fi[:], pattern=[[1, 1]], base=0, channel_multiplier=1)
        ff = pp.tile([128, 1], F32)
        V.tensor_copy(out=ff[:], in_=fi[:])
        s1 = pp.tile([128, 1], F32)
        s2 = pp.tile([128, 1], F32)
        i1 = pp.tile([128, 1], I32)
        V.tensor_scalar(out=s1[:], in0=ff[:], scalar1=1.0 / 32, scalar2=None, op0=AO.mult)
        V.tensor_copy(out=i1[:], in_=s1[:])
        V.tensor_copy(out=s2[:], in_=i1[:])          # rho = p//32
        s3 = pp.tile([128, 1], F32)
        V.tensor_tensor(out=s3[:], in0=s2[:], in1=s1[:], op=AO.is_gt)
        V.tensor_tensor(out=s2[:], in0=s2[:], in1=s3[:], op=AO.subtract)
        fmod = pp.tile([128, 1], F32)
        V.scalar_tensor_tensor(out=fmod[:], in0=s2[:], scalar=-32.0, in1=ff[:],
                               op0=AO.mult, op1=AO.add)   # f = p%32
        sc0 = pp.tile([128, 1], F32)
        V.tensor_scalar(out=sc0[:], in0=fmod[:], scalar1=-1.0, scalar2=-3.0,
                        op0=AO.mult, op1=AO.add)          # -3 - f
        rho_f = s2
        rho1_f = pp.tile([128, 1], F32)                   # (rho+1)%4
        V.tensor_scalar(out=rho1_f[:], in0=rho_f[:], scalar1=1.0, scalar2=None, op0=AO.add)
        w4t = pp.tile([128, 1], F32)
        V.tensor_scalar(out=w4t[:], in0=rho1_f[:], scalar1=4.0, scalar2=None, op0=AO.is_ge)
        V.scalar_tensor_tensor(out=rho1_f[:], in0=w4t[:], scalar=-4.0, in1=rho1_f[:],
                               op0=AO.mult, op1=AO.add)

        itc = pp.tile([128, NCH], I32)
        G.iota(itc[:], pattern=[[0, NCH // 4], [1, 4]], base=3,
               channel_multiplier=0)
        tcol3 = pp.tile([128, NCH], F32)                  # c%4 + 3
        V.tensor_copy(out=tcol3[:], in_=itc[:])
        itc2 = pp.tile([128, NCH], I32)
        G.iota(itc2[:], pattern=[[0, NCH // GS], [32, GS]], base=31 - 20000,
               channel_multiplier=0)
        S_c = pp.tile([128, NCH], F32)                    # 32*(c%GS) + 31 - 20000
        V.tensor_copy(out=S_c[:], in_=itc2[:])
        # EOFF: moves t=1 chunks' source-(c-1) taps (all valid taps at rho==0,
        # c%4==1) into the per-group extra scatter region: u16 offset from the
        # main base 32*(c%GS): s=1 -> 256-32 = +224, s=5 -> 256+32-160 = +128.
        itc3 = pp.tile([128, NCH], I32)
        G.iota(itc3[:], pattern=[[0, NCH // GS], [1, GS]], base=0,
               channel_multiplier=0)
        c8f = pp.tile([128, NCH], F32)
        V.tensor_copy(out=c8f[:], in_=itc3[:])
        e1 = pp.tile([128, NCH], F32)
        e5 = pp.tile([128, NCH], F32)
        V.tensor_scalar(out=e1[:], in0=c8f[:], scalar1=1.0, scalar2=224.0,
                        op0=AO.is_equal, op1=AO.mult)
        V.tensor_scalar(out=e5[:], in0=c8f[:], scalar1=5.0, scalar2=128.0,
                        op0=AO.is_equal, op1=AO.mult)
        EOFF = pp.tile([128, NCH], F32)
        V.memset(EOFF[:], 0.0)
        V.tensor_tensor(out=EOFF[0:32, :], in0=e1[0:32, :], in1=e5[0:32, :],
                        op=AO.add)

        # ------- natural [NP,128] -> replicated S-plane [128,NCH] -------
        # pairs: 0..1 = zf (fp32); T_blk holds the 14 blk pairs in bf16
        T_all = pp.tile([128, 2 * 128], F32)
        T_blk = pp.tile([128, 14 * 128], BF16)

        zfR = [pp.tile([128, NCH], F32, name=f"zfR{b}", tag=f"zfR{b}")
               for b in range(B_LOC)]
        blkR = [pp.tile([128, NCH, 7], BF16, name=f"blkR{b}", tag=f"blkR{b}")
                for b in range(B_LOC)]
        xI = pp.tile([128, 2 * NP], F32)   # transposed x, col 2*P+... seq-interleaved

        def nat_transpose(src_ap, k, engine, bf=False):
            """[NP,128] natural block -> (T_blk if bf else T_all) pair k."""
            dt = BF16 if bf else F32
            ps = psum_t.tile([128, 128], dt, name="ps_t", tag="ps_t")
            nc.tensor.transpose(ps[:], src_ap, (identb if bf else ident)[:])
            dst_t = T_blk if bf else T_all
            dst = dst_t[:, 128 * k:128 * (k + 1)]
            if engine == 0:
                V.tensor_copy(out=dst, in_=ps[:])
            else:
                SC.activation(out=dst, in_=ps[:],
                              func=AF.Copy, bias=0.0, scale=1.0)

        def rep_evac(kk, rho, psR, off, engine, W=128, P0=0):
            """psR[:, W*off + ...] -> strided S-plane columns P0..P0+W for
            pair kk.  ACT takes the low chunk range (needed first by the
            chain), DVE the high range (long deadline)."""
            b, q = kk % 2, kk // 2
            if q == 0:
                dstF = zfR[b][:].rearrange("p (P r) -> p P r", r=4)
                dst = dstF[:, P0:P0 + W, rho]
            else:
                dstF = blkR[b][:].rearrange("p (P r) j -> p P r j", r=4)
                dst = dstF[:, P0:P0 + W, rho, q - 1]
            src = psR[:, W * off:W * (off + 1)]
            if W <= 8:
                SC.activation(out=dst, in_=src, func=AF.Copy, bias=0.0,
                              scale=1.0)
            else:
                half = (64 - P0) if P0 < 64 else 0
                if half > 0:
                    SC.activation(out=dst[:, 0:half], in_=src[:, 0:half],
                                  func=AF.Copy, bias=0.0, scale=1.0)
                if half < W:
                    V.tensor_copy(out=dst[:, half:W], in_=src[:, half:W])

        # zf transposes first (pairs k=0,1), then replicate -> zfR early
        for b in range(B_LOC):
            nat_transpose(zf[:, b * 128:(b + 1) * 128], b, engine=1)
        for rho in range(4):
            psR = psum_r.tile([128, 256], F32, name="ps_rz", tag="ps_t")
            nc.tensor.matmul(psR[:], Rep[rho][:], T_all[:, 0:256],
                             start=True, stop=True)
            for kk in range(2):
                rep_evac(kk, rho, psR, kk, engine=1)

        # remaining tap math on DVE while zf replication + index math proceed
        D = f0c
        V.tensor_tensor(out=D[:], in0=f0c[:], in1=zf[:], op=AO.subtract)
        u = [pp.tile([NP, 256], F32, name=f"u{m}", tag=f"u{m}") for m in range(6)]
        for m in range(6):
            V.tensor_scalar(out=u[m][:], in0=D[:], scalar1=float(2 - m),
                            scalar2=None, op0=AO.add)
        pre = [None] * 6
        suf = [None] * 7
        pre[1] = u[0]
        for j in range(2, 6):
            pre[j] = pp.tile([NP, 256], F32, name=f"pre{j}", tag=f"pre{j}")
            V.tensor_tensor(out=pre[j][:], in0=pre[j - 1][:], in1=u[j - 1][:], op=AO.mult)
        suf[5] = u[5]
        for j in range(4, 0, -1):
            suf[j] = pp.tile([NP, 256], F32, name=f"suf{j}", tag=f"suf{j}")
            V.tensor_tensor(out=suf[j][:], in0=suf[j + 1][:], in1=u[j][:], op=AO.mult)
        w = [pp.tile([NP, 256], F32, name=f"w{j}", tag=f"w{j}") for j in range(6)]
        V.tensor_scalar(out=w[0][:], in0=suf[1][:], scalar1=INV_D[0], scalar2=None, op0=AO.mult)
        for j in range(1, 5):
            V.scalar_tensor_tensor(out=w[j][:], in0=pre[j][:], scalar=INV_D[j],
                                   in1=suf[j + 1][:], op0=AO.mult, op1=AO.mult)
        V.tensor_scalar(out=w[5][:], in0=pre[5][:], scalar1=INV_D[5], scalar2=None, op0=AO.mult)

        blk = [pp.tile([NP, 256], BF16, name=f"blk{j}", tag=f"blk{j}") for j in range(7)]
        V.tensor_tensor(out=blk[0][:], in0=b0t[:], in1=w[0][:], op=AO.mult)
        for j in range(1, 6):
            V.tensor_tensor(out=tmp2[:], in0=b0t[:], in1=w[j][:], op=AO.mult)
            V.tensor_tensor(out=tmp1[:], in0=t_gp[:], in1=w[j - 1][:], op=AO.mult)
            V.tensor_tensor(out=blk[j][:], in0=tmp2[:], in1=tmp1[:], op=AO.add)
        V.tensor_tensor(out=blk[6][:], in0=t_gp[:], in1=w[5][:], op=AO.mult)

        # blk + x transposes
        for j in range(7):
            for b in range(B_LOC):
                nat_transpose(blk[j][:, b * 128:(b + 1) * 128], 2 * j + b,
                              engine=1, bf=True)
        for b in range(B_LOC):
            ps = psum_t.tile([128, 128], F32, name="ps_x", tag="ps_t")
            nc.tensor.transpose(ps[:], nat_x[:, b * 128:(b + 1) * 128], ident[:])
            V.tensor_copy(out=xI[:].rearrange("p (P s) -> p P s", s=2)[:, :, b],
                          in_=ps[:])
        # replicate blk planes (bf16, 1 cycle/row on the PE) in two passes:
        # pass 1 covers P<8 (chunks<32) so the first scatter groups unblock
        # ~35us earlier; pass 2 fills P>=8.
        for P0, PW in ((0, 8), (8, 120)):
            for rho in range(4):
                for col0, npair in ((0, 4), (512, 4), (1024, 4), (1536, 2)):
                    Tg = (T_blk[:, col0:col0 + 128 * npair]
                          .rearrange("p (k P) -> p k P", P=128)[:, :, P0:P0 + PW])
                    psR = psum_r.tile([128, PW * npair], F32, name="ps_rb",
                                      tag="ps_t")
                    nc.tensor.matmul(psR[:], Repb[rho][:], Tg,
                                     start=True, stop=True)
                    for kk in range(npair):
                        rep_evac(2 + col0 // 128 + kk, rho, psR, kk,
                                 engine=1, W=PW, P0=P0)

        # ---------------- scatter index computation (GPSIMD) ----------------
        # v0 = zf - 3 - f; fl = v0//32; m = v0%32; uB = c%4 + 3 - fl
        # no-wrap valid: (uB%4 == rho); wrap valid: (uB%4 == (rho+1)%4)
        # idx[j] = valid_j*20000 + 32*iw_j + 31 - m - j - 20000 + 32*(c%GS)
        idxR = [pp.tile([128, NCH, 7], I16, name=f"idxR{b}", tag=f"idxR{b}")
                for b in range(B_LOC)]
        gv0 = pp.tile([128, NCH], F32)
        gtA = pp.tile([128, NCH], F32)
        gfl_i = pp.tile([128, NCH], I16)
        gflf = pp.tile([128, NCH], F32)
        gm0 = pp.tile([128, NCH], F32)
        guB = pp.tile([128, NCH], F32)
        gw4 = pp.tile([128, NCH], F32)
        givA = pp.tile([128, NCH], F32)
        givB = pp.tile([128, NCH], F32)
        gbase = pp.tile([128, NCH], F32)
        gdv = pp.tile([128, NCH], F32)
        gdiff = pp.tile([128, NCH], F32)
        gidxA = pp.tile([128, NCH], F32)
        giw = pp.tile([128, NCH], F32)
        gt = pp.tile([128, NCH], F32)
        gtj = pp.tile([128, NCH], F32)

        NSL = 4                       # column slices for early scatter start
        SW = NCH // NSL

        def idx_math(b, sl, sink=None):
            def emit(f, j=None):
                if sink is None:
                    f(j)
                else:
                    sink.append(lambda jj=j: f(jj))
            cs = slice(sl * SW, (sl + 1) * SW)
            emit(lambda _j: (
                V.tensor_scalar(out=gv0[:, cs], in0=zfR[b][:, cs], scalar1=sc0[:],
                                scalar2=None, op0=AO.add)
            ))
            emit(lambda _j: (
                V.tensor_scalar(out=gtA[:, cs], in0=gv0[:, cs], scalar1=1.0 / 32,
                                scalar2=None, op0=AO.mult)
            ))
            emit(lambda _j: (
                V.tensor_copy(out=gfl_i[:, cs], in_=gtA[:, cs])
            ))
            emit(lambda _j: (
                V.tensor_copy(out=gflf[:, cs], in_=gfl_i[:, cs])
            ))
            emit(lambda _j: (
                V.tensor_tensor(out=gw4[:, cs], in0=gflf[:, cs], in1=gtA[:, cs],
                                op=AO.is_gt)
            ))
            emit(lambda _j: (
                V.tensor_tensor(out=gflf[:, cs], in0=gflf[:, cs], in1=gw4[:, cs],
                                op=AO.subtract)
            ))
            emit(lambda _j: (
                V.scalar_tensor_tensor(out=gm0[:, cs], in0=gflf[:, cs], scalar=-32.0,
                                       in1=gv0[:, cs], op0=AO.mult, op1=AO.add)
            ))
            emit(lambda _j: (
                V.scalar_tensor_tensor(out=guB[:, cs], in0=gflf[:, cs], scalar=-1.0,
                                       in1=tcol3[:, cs], op0=AO.mult, op1=AO.add)
            ))
            emit(lambda _j: (
                V.tensor_scalar(out=gw4[:, cs], in0=guB[:, cs], scalar1=4.0,
                                scalar2=None, op0=AO.is_ge)
            ))
            emit(lambda _j: (
                V.scalar_tensor_tensor(out=guB[:, cs], in0=gw4[:, cs], scalar=-4.0,
                                       in1=guB[:, cs], op0=AO.mult, op1=AO.add)
            ))
            emit(lambda _j: (
                V.tensor_scalar(out=givA[:, cs], in0=guB[:, cs], scalar1=rho_f[:],
                                scalar2=None, op0=AO.is_equal)
            ))
            emit(lambda _j: (
                V.tensor_scalar(out=givB[:, cs], in0=guB[:, cs], scalar1=rho1_f[:],
                                scalar2=None, op0=AO.is_equal)
            ))
            emit(lambda _j: (
                V.scalar_tensor_tensor(out=gbase[:, cs], in0=gm0[:, cs], scalar=-1.0,
                                       in1=S_c[:, cs], op0=AO.mult, op1=AO.add)
            ))
            emit(lambda _j: (
                V.tensor_tensor(out=gbase[:, cs], in0=gbase[:, cs],
                                in1=EOFF[:, cs], op=AO.add)
            ))
            emit(lambda _j: (
                V.tensor_tensor(out=gdv[:, cs], in0=givB[:, cs], in1=givA[:, cs],
                                op=AO.subtract)
            ))
            emit(lambda _j: (
                V.tensor_scalar(out=gdiff[:, cs], in0=gdv[:, cs], scalar1=20000.0,
                                scalar2=32.0, op0=AO.mult, op1=AO.add)
            ))
            emit(lambda _j: (
                V.scalar_tensor_tensor(out=gidxA[:, cs], in0=givA[:, cs], scalar=20000.0,
                                       in1=gbase[:, cs], op0=AO.mult, op1=AO.add)
            ))
            for j in range(7):
                emit(lambda j: (
                    V.tensor_scalar(out=giw[:, cs], in0=gm0[:, cs],
                                    scalar1=float(32 - j), scalar2=None, op0=AO.is_ge)
                ), j)
                emit(lambda _j: (
                    V.tensor_tensor(out=gt[:, cs], in0=giw[:, cs], in1=gdiff[:, cs],
                                    op=AO.mult)
                ))
                emit(lambda j: (
                    V.scalar_tensor_tensor(out=idxR[b][:, cs, j], in0=gt[:, cs],
                                           scalar=float(-j), in1=gidxA[:, cs],
                                           op0=AO.add, op1=AO.add)
                ), j)

        # -------------- interleaved ring + chain ----------------
        NRC = NCH // 4 + 1
        ringI = pp.tile([128, 2 * NRC], BF16)
        V.memset(ringI[:], 0.0)

        blkR_u16 = [blkR[b][:].bitcast(U16) for b in range(B_LOC)]

        traws = {}
        t4s_map = {}

        def scatter_group(g):
            pair = []
            for b in range(B_LOC):
                traw = traw_pool.tile([128, 32 * (GS + 2)], BF16, name="traw",
                                      tag=f"traw{b}")
                G.local_scatter(
                    out_ap=traw[:].bitcast(U16),
                    data_ap=blkR_u16[b][:, GS * g:GS * (g + 1), :]
                    .rearrange("p c j -> p (c j)"),
                    idxs_ap=idxR[b][:, GS * g:GS * (g + 1), :]
                    .rearrange("p c j -> p (c j)"),
                    channels=128, num_elems=32 * (GS + 2), num_idxs=7 * GS,
                )
                pair.append(traw)
            traws[g] = pair
            t4s_map[g] = [t4_pool.tile([128, 32 * (GS + 2)], BF16, name="t4",
                                       tag=f"t4{b}") for b in range(B_LOC)]

        HW = 32 * (GS + 2) // 2  # transpose half-width (multiple of 32)

        def transpose_half(g, b, h):
            cs = slice(h * HW, (h + 1) * HW)
            nc.vector.transpose(out=t4s_map[g][b][:, cs], in_=traws[g][b][:, cs])

        for b in range(B_LOC):
            idx_math(b, 0)
        pending = []
        for sl in range(1, NSL):
            for b in range(B_LOC):
                idx_math(b, sl, sink=pending)
        pending.reverse()

        for g0 in range(2):
            scatter_group(g0)
            for b in range(B_LOC):
                for h in range(2):
                    transpose_half(g0, b, h)

        # incremental output transpose: segment k (ynat rows 32k..32k+32,
        # i.e. ring groups 32k..32k+32) is transposed+copied as soon as its
        # ring columns are final, so only the last segment remains at the end
        ynat = [pp.tile([NP, 128], F32, name=f"ynat{b}", tag=f"ynat{b}")
                for b in range(B_LOC)]

        def emit_out_segment(k, b, g0):
            src = (ringI[32 * g0:32 * g0 + 32, 2 + 64 * k:2 + 64 * k + 64]
                   .rearrange("p (P s) -> p P s", s=2)[:, :, b])
            ps = psum_t.tile([128, 32], BF16, name="ps_o", tag="ps_t")
            nc.tensor.transpose(ps[32 * k:32 * k + 32, :], src,
                                ident4b[32 * g0:32 * g0 + 32, :],
                                tile_position=(32 * g0, 32 * k))
            dst = ynat[b][32 * k:32 * k + 32, 32 * g0:32 * g0 + 32]
            if b == 0:
                V.tensor_copy(out=dst, in_=ps[32 * k:32 * k + 32, :])
            else:
                SC.activation(out=dst, in_=ps[32 * k:32 * k + 32, :],
                              func=AF.Copy, bias=0.0, scale=1.0)

        def emit_out_dma(k, b):
            nc.sync.dma_start(
                out=out_d[b].rearrange("(P j) -> P j", j=128)[32 * k:32 * k + 32, :],
                in_=ynat[b][32 * k:32 * k + 32, :],
            )

        for g in range(NG):
            t4s = t4s_map.pop(g)
            for s in range(GS):
                c = GS * g + s
                gg, t = c // 4, c % 4
                for b in range(B_LOC):
                    ps = psum_c.tile([128, 1], F32, name=f"ps{b}", tag=f"ps{b}")
                    pieces = PIECES[t]
                    for i, (rb, rs, cd, cb) in enumerate(pieces):
                        rows = slice(rb, rb + rs)
                        nu = gg + cd
                        col0 = (32 * GS + 32 * ((s - 1) // 4)
                                if cb == "X" else 32 * s)
                        nc.tensor.matmul(
                            ps[32 * t:32 * t + 32, 0:1],
                            t4s[b][rows, col0:col0 + 32],
                            ringI[rows, 2 * nu + b:2 * nu + b + 1],
                            start=(i == 0), stop=(i == len(pieces) - 1),
                            tile_position=(rb, 32 * t),
                        )
                    ring_dst = ringI[32 * t:32 * t + 32,
                                     2 * (1 + gg) + b:2 * (1 + gg) + b + 1]
                    x_src = xI[32 * t:32 * t + 32, 2 * gg + b:2 * gg + b + 1]
                    # alternate evac engine per chunk so each chain averages
                    # the fast-DVE and slow-ACT round-trip latencies
                    if (b + c) % 2 == 0:
                        V.scalar_tensor_tensor(
                            out=ring_dst, in0=ps[32 * t:32 * t + 32, 0:1],
                            scalar=1.0, in1=x_src, op0=AO.mult, op1=AO.add,
                        )
                    else:
                        SC.add(out=ring_dst, in_=ps[32 * t:32 * t + 32, 0:1],
                               add=x_src)
                if pending:
                    pending.pop()()
                gn = g + 2
                if gn < NG:
                    if s == 1:
                        scatter_group(gn)
                    elif s in (3, 4, 5, 6):
                        bb, hh = divmod(s - 3, 2)
                        transpose_half(gn, bb, hh)
                if g % 16 == 0 and g >= 16:
                    emit_out_segment(g // 16 - 1, s % 2, s // 2)
                elif g % 16 == 1 and g >= 16 and s < 2:
                    emit_out_dma(g // 16 - 1, s)

        # ---------------- final output segment + store ----------------
        for b in range(B_LOC):
            for g0 in range(4):
                emit_out_segment(3, b, g0)
        for b in range(B_LOC):
            emit_out_dma(3, b)


def build_program(N=N_FULL):
    nc = bacc.Bacc("TRN2", target_bir_lowering=False, debug=False,
                   enable_asserts=False)
    f0_d = nc.dram_tensor("f0", [B_LOC, N], F32, kind="ExternalInput").ap()
    x_d = nc.dram_tensor("x", [B_LOC, N], F32, kind="ExternalInput").ap()
    lb_d = nc.dram_tensor("l_b", [B_LOC, N, 2], F32, kind="ExternalInput").ap()
    out_d = nc.dram_tensor("out", [B_LOC, N], F32, kind="ExternalOutput").ap()
    with tile.TileContext(nc) as tc:
        build_kernel(tc, out_d, f0_d, x_d, lb_d, N)
    nc.compile()
    return nc


_PROGRAM_CACHE = {}


def _get_program(N=N_FULL):
    if N not in _PROGRAM_CACHE:
        _PROGRAM_CACHE[N] = build_program(N)
    return _PROGRAM_CACHE[N]


def kernel(f0, x, l_b, K=108, **kwargs):
    """Full-input entry point: shards batch across 8 cores, returns full output."""
    f0 = np.asarray(f0, dtype=np.float32)
    x = np.asarray(x, dtype=np.float32)
    l_b = np.asarray(l_b, dtype=np.float32)
    B, N = x.shape
    assert B == B_FULL and int(K) == 108
    nc = _get_program(N)
    in_maps = []
    for i in range(NCORES):
        sl = slice(i * B_LOC, (i + 1) * B_LOC)
        in_maps.append({
            "f0": np.ascontiguousarray(f0[sl]),
            "x": np.ascontiguousarray(x[sl]),
            "l_b": np.ascontiguousarray(l_b[sl]),
        })
    res = bass_utils.run_bass_kernel_spmd(nc, in_maps, core_ids=list(range(NCORES)))
    out = np.concatenate([res.results[i]["out"] for i in range(NCORES)], axis=0)
    return out.astype(np.float32)

